# revision 13
# baseline (speedup 1.0000x reference)
"""Trainium2 Bass kernel for nn_BLCD_Loss (retrieval kNN hinge loss) — v3.

Math (reference):
  yin = l2norm(yi), yit = l2norm(yi_t)
  top-16 neighbors of each yin_i among all yin_j (by cosine sim s = yin yinT)
  e1 = sum_k relu((dis(yin_i,yj_k) - dis(yit_i,yj_k))^2 - T)
  e2 = sum relu(dis(yin_i,yit_i) + M - dis(yin_i,yj_0))

Kernel strategy (8 cores, SPMD), "PACK3":
  Each core owns 1024 rows (host rotates yi so the self-diagonal block is in
  column chunk 0 on every core).  Per 128-row tile and 1024-column chunk the
  PE computes TWO bf16 matmuls into PSUM:
    s = yin_loc @ yinT
    c = 4096*s - t   (accumulated: lhs 4096*yin_loc then lhs -yit_loc;
                      4096*x and -x are exact in bf16, so c is an exact
                      function of the same hardware products as s and t)
  The DVE runs two top-16 value chains (per-1024-chunk max8 -> 64 candidates
  -> max8 + match_replace + max8): one on s, one on c.  Both rank almost
  identically (t/4096 perturbation), so rank-k pairs recover the neighbor
  t-similarity exactly: t_k = 4096*s_k - c_k.  The hinge then runs on tiny
  [128,16] arrays - no full-row sqrt, mask, hinge, or gather passes at all.
  Head: normalize rows (ACT square+Sqrt, DVE reduce+recip, Pool scale to
  bf16), transpose via bf16 identity matmul on the PE, sharing the ps_s PSUM
  ring; head groups 1..7 stream inside tile-0's chunk loop just ahead of
  the chunks that consume them.
  Selection fidelity & rank-pairing validated offline on the fixed dataset
  (e1 rel err 1.6e-4, e2 2.7e-7, total 4.2e-5).
"""

import numpy as np

N, D = 8192, 128
NCORES = 8
ROWS = N // NCORES          # 1024 rows per core
NRT = ROWS // 128           # 8 row-tiles per core
CH = 1024                   # PSUM chunk width (2 banks)
NCH = N // CH               # 8 chunks per row-tile
T_THR = 0.0025
MARGIN = 0.5
EPS = 1e-12
C0 = 0.5 + 0.25e-12         # dis = sqrt(s*(-0.5) + C0)
PACK_A = 4096.0             # c = A*s - t  (power of two: exact in bf16)
KNOCK_S = 16.0              # diagonal knock on s (diag -> ~-15)
KNOCK_C = PACK_A * KNOCK_S  # diagonal knock on c (knocks cancel in t_ii)
NEG = -1.0e30               # match_replace fill

_CACHE = {}


def _build_module():
    import os
    import concourse.bass as bass  # noqa: F401
    import concourse.tile as tile
    from contextlib import ExitStack
    from concourse import bacc, mybir

    STAGE = int(os.environ.get("BLCD_STAGE", "5"))
    import os as _os

    f32 = mybir.dt.float32
    bf16 = mybir.dt.bfloat16
    AF = mybir.ActivationFunctionType
    ALU = mybir.AluOpType
    AX = mybir.AxisListType

    nc = bacc.Bacc("TRN2", target_bir_lowering=False, debug=False,
                   num_devices=NCORES)

    yi_d = nc.dram_tensor("yi_rot", [N, D], f32, kind="ExternalInput")
    yit_d = nc.dram_tensor("yit_loc", [ROWS, D], f32, kind="ExternalInput")
    eye_d = nc.dram_tensor("eye1", [128, 128], f32, kind="ExternalInput")
    eyek_d = nc.dram_tensor("eyek", [128, 128], f32, kind="ExternalInput")
    eyekc_d = nc.dram_tensor("eyekc", [128, 128], f32, kind="ExternalInput")
    out_d = nc.dram_tensor("out", [128, 2 * NRT], f32, kind="ExternalOutput")
    DBG = os.environ.get("BLCD_DBG") == "1"
    if DBG:
        dbg_sk = nc.dram_tensor("dbg_sk", [128, 128], f32, kind="ExternalOutput")
        dbg_ck = nc.dram_tensor("dbg_ck", [128, 128], f32, kind="ExternalOutput")
        dbg_cs = nc.dram_tensor("dbg_cs", [128, 64], f32, kind="ExternalOutput")
        dbg_cc = nc.dram_tensor("dbg_cc", [128, 64], f32, kind="ExternalOutput")
        dbg_ps = nc.dram_tensor("dbg_ps", [128, 1024], f32, kind="ExternalOutput")
        dbg_pc = nc.dram_tensor("dbg_pc", [128, 1024], f32, kind="ExternalOutput")

    yi_r = yi_d.ap().rearrange("(n p) d -> p n d", p=128)     # [128, 64, 128]
    yit_r = yit_d.ap().rearrange("(n p) d -> p n d", p=128)   # [128, 8, 128]

    with tile.TileContext(nc) as tc, ExitStack() as ctx:
        cpool = ctx.enter_context(tc.tile_pool(name="consts", bufs=1))
        ppool = ctx.enter_context(tc.tile_pool(name="persist", bufs=1))
        smpool = ctx.enter_context(tc.tile_pool(name="small", bufs=4))

        eye = cpool.tile([128, 128], f32)
        eyek = cpool.tile([128, 128], f32)
        eyekc = cpool.tile([128, 128], f32)
        eyeb = cpool.tile([128, 128], bf16)
        knkb_s = cpool.tile([128, 128], bf16)
        knkb_c = cpool.tile([128, 128], bf16)
        c0b = cpool.tile([128, 1], f32)
        nc.gpsimd.memset(c0b[:], C0)
        epsb = cpool.tile([128, 1], f32)
        nc.gpsimd.memset(epsb[:], EPS)
        epsqb = cpool.tile([128, 1], f32)
        nc.gpsimd.memset(epsqb[:], EPS / 4.0)

        yinT = ppool.tile([128, N], bf16)       # normalized yi, transposed
        yinTA = ppool.tile([128, ROWS], bf16)   # 4096 * yinT local block
        yitTn = ppool.tile([128, ROWS], bf16)   # -normalized yi_t, transposed
        e1acc = ppool.tile([128, NRT], f32)
        e2acc = ppool.tile([128, NRT], f32)
        nc.gpsimd.memset(e1acc[:], 0.0)
        nc.gpsimd.memset(e2acc[:], 0.0)
        dis_td = ppool.tile([128, NRT], f32)    # dis(yin_i, yit_i) per tile

        n_rt = NRT if STAGE >= 5 else int(os.environ.get("BLCD_NRT", "1"))
        with tc.tile_pool(name="headbig", bufs=4) as hbig, \
             tc.tile_pool(name="headrows", bufs=10) as hrows, \
             tc.tile_pool(name="headkeep", bufs=1) as hkeep, \
             tc.tile_pool(name="headsm", bufs=4) as hsm, \
             tc.tile_pool(name="cands", bufs=2) as candp, \
             tc.tile_pool(name="ps_s", bufs=2, space="PSUM") as ps_spool, \
             tc.tile_pool(name="ps_c", bufs=2, space="PSUM") as ps_cpool:

            def fetch_rows(src_r, g):
                rows = hrows.tile([128, 8, 128], f32, tag="rows")
                nc.sync.dma_start(rows[:], src_r[:, g:g + 8, :])
                return rows

            def emit_head_group(rows, g, dsts, keep=False,
                                evict_dve=False):
                """dsts: list of (dstT, scale_mode); scale_mode in
                {'pos','neg','4k'} applied via the per-row rinv variant.
                Returns the rows_n bf16 tile of the last dst."""
                sq = hsm.tile([128, 8], f32, tag="sq")
                sqscr = hbig.tile([128, 8, 128], f32, tag="sqscr")
                nc.scalar.activation(
                    sqscr[:].rearrange("p a b -> p (a b)"),
                    rows[:].rearrange("p a b -> p (a b)"), AF.Square)
                nc.vector.tensor_reduce(sq[:], sqscr[:], op=ALU.add,
                                        axis=AX.X)
                nrm = hsm.tile([128, 8], f32, tag="nrm")
                nc.scalar.activation(nrm[:], sq[:], AF.Sqrt, bias=epsb[:])
                rinv = hsm.tile([128, 8], f32, tag="rinv")
                nc.vector.reciprocal(rinv[:], nrm[:])
                for (dstT, mode) in dsts:
                    if mode == 'pos':
                        rv = rinv
                    else:
                        rv = hsm.tile([128, 8], f32, tag="rv" + mode)
                        scl = -1.0 if mode == 'neg' else PACK_A
                        nc.vector.tensor_scalar(rv[:], rinv[:], scl, None,
                                                ALU.mult)
                    pool_n = hkeep if keep else hbig
                    rows_n = pool_n.tile([128, 8, 128], bf16,
                                         tag="rows_n" + mode)
                    for jj in range(8):
                        nc.gpsimd.tensor_scalar(rows_n[:, jj, :],
                                                rows[:, jj, :],
                                                rv[:, jj:jj + 1], None,
                                                ALU.mult)
                    ps = ps_spool.tile([128, CH], f32, tag="ps_s")
                    for jj in range(8):
                        nc.tensor.matmul(ps[:, jj * 128:(jj + 1) * 128],
                                         rows_n[:, jj, :], eyeb[:],
                                         start=True, stop=True)
                    if evict_dve:
                        nc.vector.tensor_copy(
                            dstT[:, g * 128:g * 128 + CH], ps[:])
                    else:
                        nc.scalar.copy(dstT[:, g * 128:g * 128 + CH], ps[:])
                return rows_n

            def emit_tail(rt, s_k, c_k):
                # tail: recover t_k, hinge (deferred one tile for overlap;
                # small SBUF-only arithmetic runs on the idle Pool engine)
                t_k = smpool.tile([128, 16], f32, tag="tk")
                nc.vector.scalar_tensor_tensor(t_k[:], s_k[:], PACK_A,
                                               c_k[:], ALU.mult, ALU.subtract)
                dis_a = smpool.tile([128, 16], f32, tag="da")
                nc.scalar.activation(dis_a[:], s_k[:], AF.Sqrt,
                                     scale=-0.5, bias=c0b[:])
                dis_b = smpool.tile([128, 16], f32, tag="db")
                nc.scalar.activation(dis_b[:], t_k[:], AF.Sqrt,
                                     scale=-0.5, bias=c0b[:])
                diff = smpool.tile([128, 16], f32, tag="df")
                nc.gpsimd.tensor_sub(diff[:], dis_a[:], dis_b[:])
                sqd = smpool.tile([128, 16], f32, tag="sqd")
                nc.gpsimd.tensor_mul(sqd[:], diff[:], diff[:])
                hng = smpool.tile([128, 16], f32, tag="hg")
                nc.gpsimd.tensor_scalar(hng[:], sqd[:], T_THR, 0.0,
                                        ALU.subtract, ALU.max)
                hs2 = smpool.tile([128, 16], f32, tag="hs2")
                nc.vector.tensor_scalar(hs2[:], hng[:], 1.0, None,
                                        ALU.mult, ALU.add,
                                        accum_out=e1acc[:, rt:rt + 1])
                # e2: dis_td + M - dis_nn, relu
                o2 = smpool.tile([128, 1], f32, tag="o2")
                nc.vector.scalar_tensor_tensor(o2[:], dis_a[:, 0:1], -1.0,
                                               dis_td[:, rt:rt + 1],
                                               ALU.mult, ALU.add)
                nc.vector.tensor_scalar(e2acc[:, rt:rt + 1], o2[:], MARGIN,
                                        0.0, ALU.add, ALU.max)

            # prefetch every row group before any compute is queued
            pre = [fetch_rows(yi_r, 0), fetch_rows(yit_r, 0)] + \
                  [fetch_rows(yi_r, g) for g in range(8, 64, 8)]
            nc.sync.dma_start(eye[:], eye_d[:])
            nc.sync.dma_start(eyek[:], eyek_d[:])
            nc.sync.dma_start(eyekc[:], eyekc_d[:])
            nc.gpsimd.tensor_copy(eyeb[:], eye[:])
            # knock matrices in bf16 (-16*eye, -65536*eye: exact in bf16)
            nc.gpsimd.tensor_scalar(knkb_s[:], eye[:], -KNOCK_S, None,
                                    ALU.mult)
            nc.gpsimd.tensor_scalar(knkb_c[:], eye[:], -KNOCK_C, None,
                                    ALU.mult)

            # group 0 covers the local block: also build the scaled lhs
            # copies (4096*yin and -yit, both exact bf16 transforms)
            rows_yi0 = emit_head_group(pre[0], 0, [(yinTA, '4k'),
                                                   (yinT, 'pos')],
                                       keep=True)
            rows_ytn = emit_head_group(pre[1], 0, [(yitTn, 'neg')],
                                       keep=True)


            pending = None
            for rt in range(n_rt):
                lhs_s = yinT[:, rt * 128:(rt + 1) * 128]
                lhs_sA = yinTA[:, rt * 128:(rt + 1) * 128]
                lhs_tn = yitTn[:, rt * 128:(rt + 1) * 128]
                cand_s = candp.tile([128, NCH * 8], f32, tag="cs")
                cand_c = candp.tile([128, NCH * 8], f32, tag="cc")
                dsl = slice(rt * 128, (rt + 1) * 128)
                for cc in range(NCH):
                    if rt == 0 and cc >= 1:
                        emit_head_group(pre[cc + 1], cc * 8,
                                        [(yinT, 'pos')])
                    ps_s = ps_spool.tile([128, CH], f32)
                    ps_c = ps_cpool.tile([128, CH], f32)
                    # chunk 0 carries the self-block: split the matmuls at
                    # the diagonal 128-column range and knock it with an
                    # extra accumulated -K*eye matmul (lhsT=I => out += rhs)
                    if cc == 0:
                        # split each 512-half at the diagonal block, never
                        # crossing a 512 boundary (PSUM banks)
                        splits = []
                        for h in range(2):
                            h0 = h * 512
                            d0 = rt * 128
                            if h0 <= d0 < h0 + 512:
                                splits += [(h0, d0 - h0, None),
                                           (d0, 128, 'knock'),
                                           (d0 + 128, h0 + 512 - d0 - 128,
                                            None)]
                            else:
                                splits.append((h0, 512, None))
                    else:
                        splits = [(0, 512, None), (512, 512, None)]
                    for (o, w, kn) in splits:
                        if w <= 0:
                            continue
                        hs = slice(o, o + w)
                        rhs = yinT[:, cc * CH + o: cc * CH + o + w]
                        nc.tensor.matmul(ps_s[:, hs], lhs_s, rhs,
                                         start=True, stop=kn is None)
                        if kn:
                            nc.tensor.matmul(ps_s[:, hs], eyeb[:], knkb_s[:],
                                             start=False, stop=True)
                        nc.tensor.matmul(ps_c[:, hs], lhs_sA, rhs,
                                         start=True, stop=False)
                        nc.tensor.matmul(ps_c[:, hs], lhs_tn, rhs,
                                         start=False, stop=kn is None)
                        if kn:
                            nc.tensor.matmul(ps_c[:, hs], eyeb[:], knkb_c[:],
                                             start=False, stop=True)
                    # per-chunk top-8 candidates
                    nc.vector.max(cand_s[:, cc * 8:(cc + 1) * 8], ps_s[:])
                    nc.vector.max(cand_c[:, cc * 8:(cc + 1) * 8], ps_c[:])
                    if DBG and rt == 0 and cc == 0:
                        scr_ps = smpool.tile([128, 1024], f32, tag="dps")
                        nc.scalar.copy(scr_ps[:], ps_s[:])
                        nc.sync.dma_start(dbg_ps[:], scr_ps[:])
                        scr_pc = smpool.tile([128, 1024], f32, tag="dpc")
                        nc.scalar.copy(scr_pc[:], ps_c[:])
                        nc.sync.dma_start(dbg_pc[:], scr_pc[:])

                if rt == 0:
                    # dis(yin_i, yit_i) per local row, from normalized rows
                    # (deferred past the head so it never stalls it):
                    # u = yin - yit; dis_td = sqrt(0.25*|u|^2 + eps/4)
                    sqtd = smpool.tile([128, NRT], f32, tag="sqtd")
                    for jj in range(NRT):
                        u_td = hbig.tile([128, 128], bf16, tag="u_td")
                        nc.gpsimd.tensor_add(u_td[:], rows_yi0[:, jj, :],
                                             rows_ytn[:, jj, :])
                        uscr = hbig.tile([128, 128], f32, tag="uscr")
                        nc.scalar.activation(uscr[:], u_td[:], AF.Square,
                                             accum_out=sqtd[:, jj:jj + 1])
                    nc.scalar.activation(dis_td[:], sqtd[:], AF.Sqrt,
                                         scale=0.25, bias=epsqb[:])

                # top-16 chains
                s_k = smpool.tile([128, 16], f32, tag="sk")
                c_k = smpool.tile([128, 16], f32, tag="ck")
                nc.vector.max(s_k[:, 0:8], cand_s[:])
                nc.vector.match_replace(cand_s[:], s_k[:, 0:8], cand_s[:], NEG)
                nc.vector.max(s_k[:, 8:16], cand_s[:])
                nc.vector.max(c_k[:, 0:8], cand_c[:])
                nc.vector.match_replace(cand_c[:], c_k[:, 0:8], cand_c[:], NEG)
                nc.vector.max(c_k[:, 8:16], cand_c[:])

                if DBG:
                    nc.sync.dma_start(dbg_sk[:, rt * 16:(rt + 1) * 16],
                                      s_k[:])
                    nc.sync.dma_start(dbg_ck[:, rt * 16:(rt + 1) * 16],
                                      c_k[:])
                if DBG and rt == 0:
                    nc.sync.dma_start(dbg_cs[:], cand_s[:])
                    nc.sync.dma_start(dbg_cc[:], cand_c[:])
                if pending is not None:
                    emit_tail(*pending)
                pending = (rt, s_k, c_k)
            if pending is not None:
                emit_tail(*pending)

        # ---------------- tail: store per-tile partials (host sums) -------
        nc.sync.dma_start(out_d[:, 0:NRT], e1acc[:])
        nc.sync.dma_start(out_d[:, NRT:2 * NRT], e2acc[:])

    nc.compile()
    return nc


def kernel(yi: np.ndarray, yi_t: np.ndarray):
    from concourse.bass_utils import run_bass_kernel_spmd

    if "nc" not in _CACHE:
        _CACHE["nc"] = _build_module()
    nc = _CACHE["nc"]

    yi = np.ascontiguousarray(np.asarray(yi, dtype=np.float32))
    yi_t = np.ascontiguousarray(np.asarray(yi_t, dtype=np.float32))
    eye1 = np.eye(128, dtype=np.float32)
    eyek = (KNOCK_S * np.eye(128)).astype(np.float32)
    eyekc = (KNOCK_C * np.eye(128)).astype(np.float32)

    in_maps = []
    for c in range(NCORES):
        lo = c * ROWS
        yi_rot = np.concatenate([yi[lo:], yi[:lo]], axis=0)
        in_maps.append({
            "yi_rot": np.ascontiguousarray(yi_rot),
            "yit_loc": np.ascontiguousarray(yi_t[lo:lo + ROWS]),
            "eye1": eye1,
            "eyek": eyek,
            "eyekc": eyekc,
        })

    res = run_bass_kernel_spmd(nc, in_maps, list(range(NCORES))).results

    e1 = np.float64(0.0)
    e2 = np.float64(0.0)
    for c in range(NCORES):
        out = res[c]["out"]
        e1 += out[:, 0:NRT].astype(np.float64).sum()
        e2 += out[:, NRT:2 * NRT].astype(np.float64).sum()
    e1 = np.float32(e1)
    e2 = np.float32(e2)
    return (np.float32(e1 + e2), e1, e2)


# revision 14
# speedup vs baseline: 1.0014x; 1.0014x over previous
"""Trainium2 Bass kernel for nn_BLCD_Loss (retrieval kNN hinge loss) — v3.

Math (reference):
  yin = l2norm(yi), yit = l2norm(yi_t)
  top-16 neighbors of each yin_i among all yin_j (by cosine sim s = yin yinT)
  e1 = sum_k relu((dis(yin_i,yj_k) - dis(yit_i,yj_k))^2 - T)
  e2 = sum relu(dis(yin_i,yit_i) + M - dis(yin_i,yj_0))

Kernel strategy (8 cores, SPMD), "PACK3":
  Each core owns 1024 rows (host rotates yi so the self-diagonal block is in
  column chunk 0 on every core).  Per 128-row tile and 1024-column chunk the
  PE computes TWO bf16 matmuls into PSUM:
    s = yin_loc @ yinT
    c = 4096*s - t   (accumulated: lhs 4096*yin_loc then lhs -yit_loc;
                      4096*x and -x are exact in bf16, so c is an exact
                      function of the same hardware products as s and t)
  The DVE runs two top-16 value chains (per-1024-chunk max8 -> 64 candidates
  -> max8 + match_replace + max8): one on s, one on c.  Both rank almost
  identically (t/4096 perturbation), so rank-k pairs recover the neighbor
  t-similarity exactly: t_k = 4096*s_k - c_k.  The hinge then runs on tiny
  [128,16] arrays - no full-row sqrt, mask, hinge, or gather passes at all.
  Head: normalize rows (ACT square+Sqrt, DVE reduce+recip, Pool scale to
  bf16), transpose via bf16 identity matmul on the PE, sharing the ps_s PSUM
  ring; head groups 1..7 stream inside tile-0's chunk loop just ahead of
  the chunks that consume them.
  Selection fidelity & rank-pairing validated offline on the fixed dataset
  (e1 rel err 1.6e-4, e2 2.7e-7, total 4.2e-5).
"""

import numpy as np

N, D = 8192, 128
NCORES = 8
ROWS = N // NCORES          # 1024 rows per core
NRT = ROWS // 128           # 8 row-tiles per core
CH = 1024                   # PSUM chunk width (2 banks)
NCH = N // CH               # 8 chunks per row-tile
T_THR = 0.0025
MARGIN = 0.5
EPS = 1e-12
C0 = 0.5 + 0.25e-12         # dis = sqrt(s*(-0.5) + C0)
PACK_A = 4096.0             # c = A*s - t  (power of two: exact in bf16)
KNOCK_S = 16.0              # diagonal knock on s (diag -> ~-15)
KNOCK_C = PACK_A * KNOCK_S  # diagonal knock on c (knocks cancel in t_ii)
NEG = -1.0e30               # match_replace fill

_CACHE = {}


def _build_module():
    import os
    import concourse.bass as bass  # noqa: F401
    import concourse.tile as tile
    from contextlib import ExitStack
    from concourse import bacc, mybir

    STAGE = int(os.environ.get("BLCD_STAGE", "5"))
    import os as _os

    f32 = mybir.dt.float32
    bf16 = mybir.dt.bfloat16
    AF = mybir.ActivationFunctionType
    ALU = mybir.AluOpType
    AX = mybir.AxisListType

    nc = bacc.Bacc("TRN2", target_bir_lowering=False, debug=False,
                   num_devices=NCORES)

    yi_d = nc.dram_tensor("yi_rot", [N, D], f32, kind="ExternalInput")
    yit_d = nc.dram_tensor("yit_loc", [ROWS, D], f32, kind="ExternalInput")
    eye_d = nc.dram_tensor("eye1", [128, 128], f32, kind="ExternalInput")
    eyek_d = nc.dram_tensor("eyek", [128, 128], f32, kind="ExternalInput")
    eyekc_d = nc.dram_tensor("eyekc", [128, 128], f32, kind="ExternalInput")
    out_d = nc.dram_tensor("out", [128, 2 * NRT], f32, kind="ExternalOutput")
    DBG = os.environ.get("BLCD_DBG") == "1"
    if DBG:
        dbg_sk = nc.dram_tensor("dbg_sk", [128, 128], f32, kind="ExternalOutput")
        dbg_ck = nc.dram_tensor("dbg_ck", [128, 128], f32, kind="ExternalOutput")
        dbg_cs = nc.dram_tensor("dbg_cs", [128, 64], f32, kind="ExternalOutput")
        dbg_cc = nc.dram_tensor("dbg_cc", [128, 64], f32, kind="ExternalOutput")
        dbg_ps = nc.dram_tensor("dbg_ps", [128, 1024], f32, kind="ExternalOutput")
        dbg_pc = nc.dram_tensor("dbg_pc", [128, 1024], f32, kind="ExternalOutput")

    yi_r = yi_d.ap().rearrange("(n p) d -> p n d", p=128)     # [128, 64, 128]
    yit_r = yit_d.ap().rearrange("(n p) d -> p n d", p=128)   # [128, 8, 128]

    with tile.TileContext(nc) as tc, ExitStack() as ctx:
        cpool = ctx.enter_context(tc.tile_pool(name="consts", bufs=1))
        ppool = ctx.enter_context(tc.tile_pool(name="persist", bufs=1))
        smpool = ctx.enter_context(tc.tile_pool(name="small", bufs=4))

        eye = cpool.tile([128, 128], f32)
        eyek = cpool.tile([128, 128], f32)
        eyekc = cpool.tile([128, 128], f32)
        eyeb = cpool.tile([128, 128], bf16)
        knkb_s = cpool.tile([128, 128], bf16)
        knkb_c = cpool.tile([128, 128], bf16)
        c0b = cpool.tile([128, 1], f32)
        nc.gpsimd.memset(c0b[:], C0)
        epsb = cpool.tile([128, 1], f32)
        nc.gpsimd.memset(epsb[:], EPS)
        epsqb = cpool.tile([128, 1], f32)
        nc.gpsimd.memset(epsqb[:], EPS / 4.0)

        yinT = ppool.tile([128, N], bf16)       # normalized yi, transposed
        yinTA = ppool.tile([128, ROWS], bf16)   # 4096 * yinT local block
        yitTn = ppool.tile([128, ROWS], bf16)   # -normalized yi_t, transposed
        eacc = ppool.tile([128, 2 * NRT], f32)   # e1 cols 0:8, e2 cols 8:16
        nc.gpsimd.memset(eacc[:], 0.0)
        dis_td = ppool.tile([128, NRT], f32)    # dis(yin_i, yit_i) per tile

        n_rt = NRT if STAGE >= 5 else int(os.environ.get("BLCD_NRT", "1"))
        with tc.tile_pool(name="headbig", bufs=4) as hbig, \
             tc.tile_pool(name="headrows", bufs=10) as hrows, \
             tc.tile_pool(name="headkeep", bufs=1) as hkeep, \
             tc.tile_pool(name="headsm", bufs=4) as hsm, \
             tc.tile_pool(name="cands", bufs=2) as candp, \
             tc.tile_pool(name="ps_s", bufs=2, space="PSUM") as ps_spool, \
             tc.tile_pool(name="ps_c", bufs=2, space="PSUM") as ps_cpool:

            def fetch_rows(src_r, g):
                rows = hrows.tile([128, 8, 128], f32, tag="rows")
                nc.sync.dma_start(rows[:], src_r[:, g:g + 8, :])
                return rows

            def emit_head_group(rows, g, dsts, keep=False,
                                evict_dve=False):
                """dsts: list of (dstT, scale_mode); scale_mode in
                {'pos','neg','4k'} applied via the per-row rinv variant.
                Returns the rows_n bf16 tile of the last dst."""
                sq = hsm.tile([128, 8], f32, tag="sq")
                sqscr = hbig.tile([128, 8, 128], f32, tag="sqscr")
                nc.scalar.activation(
                    sqscr[:].rearrange("p a b -> p (a b)"),
                    rows[:].rearrange("p a b -> p (a b)"), AF.Square)
                nc.vector.tensor_reduce(sq[:], sqscr[:], op=ALU.add,
                                        axis=AX.X)
                nrm = hsm.tile([128, 8], f32, tag="nrm")
                nc.scalar.activation(nrm[:], sq[:], AF.Sqrt, bias=epsb[:])
                rinv = hsm.tile([128, 8], f32, tag="rinv")
                nc.vector.reciprocal(rinv[:], nrm[:])
                for (dstT, mode) in dsts:
                    if mode == 'pos':
                        rv = rinv
                    else:
                        rv = hsm.tile([128, 8], f32, tag="rv" + mode)
                        scl = -1.0 if mode == 'neg' else PACK_A
                        nc.vector.tensor_scalar(rv[:], rinv[:], scl, None,
                                                ALU.mult)
                    pool_n = hkeep if keep else hbig
                    rows_n = pool_n.tile([128, 8, 128], bf16,
                                         tag="rows_n" + mode)
                    for jj in range(8):
                        nc.gpsimd.tensor_scalar(rows_n[:, jj, :],
                                                rows[:, jj, :],
                                                rv[:, jj:jj + 1], None,
                                                ALU.mult)
                    ps = ps_spool.tile([128, CH], f32, tag="ps_s")
                    for jj in range(8):
                        nc.tensor.matmul(ps[:, jj * 128:(jj + 1) * 128],
                                         rows_n[:, jj, :], eyeb[:],
                                         start=True, stop=True)
                    if evict_dve:
                        nc.vector.tensor_copy(
                            dstT[:, g * 128:g * 128 + CH], ps[:])
                    else:
                        nc.scalar.copy(dstT[:, g * 128:g * 128 + CH], ps[:])
                return rows_n

            def emit_tail(rt, s_k, c_k, dve_only=False):
                # tail: recover t_k, hinge (deferred one tile for overlap;
                # small SBUF-only arithmetic runs on the idle Pool engine)
                t_k = smpool.tile([128, 16], f32, tag="tk")
                nc.vector.scalar_tensor_tensor(t_k[:], s_k[:], PACK_A,
                                               c_k[:], ALU.mult, ALU.subtract)
                dis_a = smpool.tile([128, 16], f32, tag="da")
                nc.scalar.activation(dis_a[:], s_k[:], AF.Sqrt,
                                     scale=-0.5, bias=c0b[:])
                dis_b = smpool.tile([128, 16], f32, tag="db")
                nc.scalar.activation(dis_b[:], t_k[:], AF.Sqrt,
                                     scale=-0.5, bias=c0b[:])
                eng = nc.vector if dve_only else nc.gpsimd
                diff = smpool.tile([128, 16], f32, tag="df")
                eng.tensor_sub(diff[:], dis_a[:], dis_b[:])
                sqd = smpool.tile([128, 16], f32, tag="sqd")
                eng.tensor_mul(sqd[:], diff[:], diff[:])
                hng = smpool.tile([128, 16], f32, tag="hg")
                eng.tensor_scalar(hng[:], sqd[:], T_THR, 0.0,
                                  ALU.subtract, ALU.max)
                hs2 = smpool.tile([128, 16], f32, tag="hs2")
                nc.vector.tensor_scalar(hs2[:], hng[:], 1.0, None,
                                        ALU.mult, ALU.add,
                                        accum_out=eacc[:, rt:rt + 1])
                # e2: dis_td + M - dis_nn, relu
                o2 = smpool.tile([128, 1], f32, tag="o2")
                nc.vector.scalar_tensor_tensor(o2[:], dis_a[:, 0:1], -1.0,
                                               dis_td[:, rt:rt + 1],
                                               ALU.mult, ALU.add)
                nc.vector.tensor_scalar(eacc[:, NRT + rt:NRT + rt + 1],
                                        o2[:], MARGIN, 0.0, ALU.add, ALU.max)

            # prefetch every row group before any compute is queued
            pre = [fetch_rows(yi_r, 0), fetch_rows(yit_r, 0)] + \
                  [fetch_rows(yi_r, g) for g in range(8, 64, 8)]
            nc.sync.dma_start(eye[:], eye_d[:])
            nc.sync.dma_start(eyek[:], eyek_d[:])
            nc.sync.dma_start(eyekc[:], eyekc_d[:])
            nc.gpsimd.tensor_copy(eyeb[:], eye[:])
            # knock matrices in bf16 (-16*eye, -65536*eye: exact in bf16)
            nc.gpsimd.tensor_scalar(knkb_s[:], eye[:], -KNOCK_S, None,
                                    ALU.mult)
            nc.gpsimd.tensor_scalar(knkb_c[:], eye[:], -KNOCK_C, None,
                                    ALU.mult)

            # group 0 covers the local block: also build the scaled lhs
            # copies (4096*yin and -yit, both exact bf16 transforms)
            rows_yi0 = emit_head_group(pre[0], 0, [(yinTA, '4k'),
                                                   (yinT, 'pos')],
                                       keep=True)
            rows_ytn = emit_head_group(pre[1], 0, [(yitTn, 'neg')],
                                       keep=True)


            pending = None
            for rt in range(n_rt):
                lhs_s = yinT[:, rt * 128:(rt + 1) * 128]
                lhs_sA = yinTA[:, rt * 128:(rt + 1) * 128]
                lhs_tn = yitTn[:, rt * 128:(rt + 1) * 128]
                cand_s = candp.tile([128, NCH * 8], f32, tag="cs")
                cand_c = candp.tile([128, NCH * 8], f32, tag="cc")
                dsl = slice(rt * 128, (rt + 1) * 128)
                for cc in range(NCH):
                    if rt == 0 and cc >= 1:
                        emit_head_group(pre[cc + 1], cc * 8,
                                        [(yinT, 'pos')])
                    ps_s = ps_spool.tile([128, CH], f32)
                    ps_c = ps_cpool.tile([128, CH], f32)
                    # chunk 0 carries the self-block: split the matmuls at
                    # the diagonal 128-column range and knock it with an
                    # extra accumulated -K*eye matmul (lhsT=I => out += rhs)
                    if cc == 0:
                        # split each 512-half at the diagonal block, never
                        # crossing a 512 boundary (PSUM banks)
                        splits = []
                        for h in range(2):
                            h0 = h * 512
                            d0 = rt * 128
                            if h0 <= d0 < h0 + 512:
                                splits += [(h0, d0 - h0, None),
                                           (d0, 128, 'knock'),
                                           (d0 + 128, h0 + 512 - d0 - 128,
                                            None)]
                            else:
                                splits.append((h0, 512, None))
                    else:
                        splits = [(0, 512, None), (512, 512, None)]
                    # all ps_s matmuls first so the s-scan starts early
                    for (o, w, kn) in splits:
                        if w <= 0:
                            continue
                        hs = slice(o, o + w)
                        rhs = yinT[:, cc * CH + o: cc * CH + o + w]
                        nc.tensor.matmul(ps_s[:, hs], lhs_s, rhs,
                                         start=True, stop=kn is None)
                        if kn:
                            nc.tensor.matmul(ps_s[:, hs], eyeb[:], knkb_s[:],
                                             start=False, stop=True)
                    for (o, w, kn) in splits:
                        if w <= 0:
                            continue
                        hs = slice(o, o + w)
                        rhs = yinT[:, cc * CH + o: cc * CH + o + w]
                        nc.tensor.matmul(ps_c[:, hs], lhs_sA, rhs,
                                         start=True, stop=False)
                        nc.tensor.matmul(ps_c[:, hs], lhs_tn, rhs,
                                         start=False, stop=kn is None)
                        if kn:
                            nc.tensor.matmul(ps_c[:, hs], eyeb[:], knkb_c[:],
                                             start=False, stop=True)
                    # per-chunk top-8 candidates
                    nc.vector.max(cand_s[:, cc * 8:(cc + 1) * 8], ps_s[:])
                    nc.vector.max(cand_c[:, cc * 8:(cc + 1) * 8], ps_c[:])
                    if DBG and rt == 0 and cc == 0:
                        scr_ps = smpool.tile([128, 1024], f32, tag="dps")
                        nc.scalar.copy(scr_ps[:], ps_s[:])
                        nc.sync.dma_start(dbg_ps[:], scr_ps[:])
                        scr_pc = smpool.tile([128, 1024], f32, tag="dpc")
                        nc.scalar.copy(scr_pc[:], ps_c[:])
                        nc.sync.dma_start(dbg_pc[:], scr_pc[:])

                if rt == 0:
                    # dis(yin_i, yit_i) per local row, from normalized rows
                    # (deferred past the head so it never stalls it):
                    # u = yin - yit; dis_td = sqrt(0.25*|u|^2 + eps/4)
                    sqtd = smpool.tile([128, NRT], f32, tag="sqtd")
                    for jj in range(NRT):
                        u_td = hbig.tile([128, 128], bf16, tag="u_td")
                        nc.gpsimd.tensor_add(u_td[:], rows_yi0[:, jj, :],
                                             rows_ytn[:, jj, :])
                        uscr = hbig.tile([128, 128], f32, tag="uscr")
                        nc.scalar.activation(uscr[:], u_td[:], AF.Square,
                                             accum_out=sqtd[:, jj:jj + 1])
                    nc.scalar.activation(dis_td[:], sqtd[:], AF.Sqrt,
                                         scale=0.25, bias=epsqb[:])

                # top-16 chains
                s_k = smpool.tile([128, 16], f32, tag="sk")
                c_k = smpool.tile([128, 16], f32, tag="ck")
                nc.vector.max(s_k[:, 0:8], cand_s[:])
                nc.vector.match_replace(cand_s[:], s_k[:, 0:8], cand_s[:], NEG)
                nc.vector.max(s_k[:, 8:16], cand_s[:])
                nc.vector.max(c_k[:, 0:8], cand_c[:])
                nc.vector.match_replace(cand_c[:], c_k[:, 0:8], cand_c[:], NEG)
                nc.vector.max(c_k[:, 8:16], cand_c[:])

                if DBG:
                    nc.sync.dma_start(dbg_sk[:, rt * 16:(rt + 1) * 16],
                                      s_k[:])
                    nc.sync.dma_start(dbg_ck[:, rt * 16:(rt + 1) * 16],
                                      c_k[:])
                if DBG and rt == 0:
                    nc.sync.dma_start(dbg_cs[:], cand_s[:])
                    nc.sync.dma_start(dbg_cc[:], cand_c[:])
                if pending is not None:
                    emit_tail(*pending)
                pending = (rt, s_k, c_k)
            if pending is not None:
                emit_tail(*pending, dve_only=True)

        # ---------------- tail: one DMA of all partials (host sums) ------
        nc.sync.dma_start(out_d[:], eacc[:])

    nc.compile()
    return nc


def kernel(yi: np.ndarray, yi_t: np.ndarray):
    from concourse.bass_utils import run_bass_kernel_spmd

    if "nc" not in _CACHE:
        _CACHE["nc"] = _build_module()
    nc = _CACHE["nc"]

    yi = np.ascontiguousarray(np.asarray(yi, dtype=np.float32))
    yi_t = np.ascontiguousarray(np.asarray(yi_t, dtype=np.float32))
    eye1 = np.eye(128, dtype=np.float32)
    eyek = (KNOCK_S * np.eye(128)).astype(np.float32)
    eyekc = (KNOCK_C * np.eye(128)).astype(np.float32)

    in_maps = []
    for c in range(NCORES):
        lo = c * ROWS
        yi_rot = np.concatenate([yi[lo:], yi[:lo]], axis=0)
        in_maps.append({
            "yi_rot": np.ascontiguousarray(yi_rot),
            "yit_loc": np.ascontiguousarray(yi_t[lo:lo + ROWS]),
            "eye1": eye1,
            "eyek": eyek,
            "eyekc": eyekc,
        })

    res = run_bass_kernel_spmd(nc, in_maps, list(range(NCORES))).results

    e1 = np.float64(0.0)
    e2 = np.float64(0.0)
    for c in range(NCORES):
        out = res[c]["out"]
        e1 += out[:, 0:NRT].astype(np.float64).sum()
        e2 += out[:, NRT:2 * NRT].astype(np.float64).sum()
    e1 = np.float32(e1)
    e2 = np.float32(e2)
    return (np.float32(e1 + e2), e1, e2)


# revision 15
# speedup vs baseline: 1.0298x; 1.0283x over previous
"""Trainium2 Bass kernel for nn_BLCD_Loss (retrieval kNN hinge loss) — v3.

Math (reference):
  yin = l2norm(yi), yit = l2norm(yi_t)
  top-16 neighbors of each yin_i among all yin_j (by cosine sim s = yin yinT)
  e1 = sum_k relu((dis(yin_i,yj_k) - dis(yit_i,yj_k))^2 - T)
  e2 = sum relu(dis(yin_i,yit_i) + M - dis(yin_i,yj_0))

Kernel strategy (8 cores, SPMD), "PACK3":
  Each core owns 1024 rows (host rotates yi so the self-diagonal block is in
  column chunk 0 on every core).  Per 128-row tile and 1024-column chunk the
  PE computes TWO bf16 matmuls into PSUM:
    s = yin_loc @ yinT
    c = 4096*s - t   (accumulated: lhs 4096*yin_loc then lhs -yit_loc;
                      4096*x and -x are exact in bf16, so c is an exact
                      function of the same hardware products as s and t)
  The DVE runs two top-16 value chains (per-1024-chunk max8 -> 64 candidates
  -> max8 + match_replace + max8): one on s, one on c.  Both rank almost
  identically (t/4096 perturbation), so rank-k pairs recover the neighbor
  t-similarity exactly: t_k = 4096*s_k - c_k.  The hinge then runs on tiny
  [128,16] arrays - no full-row sqrt, mask, hinge, or gather passes at all.
  Head: normalize rows (ACT square+Sqrt, DVE reduce+recip, Pool scale to
  bf16), transpose via bf16 identity matmul on the PE, sharing the ps_s PSUM
  ring; head groups 1..7 stream inside tile-0's chunk loop just ahead of
  the chunks that consume them.
  Selection fidelity & rank-pairing validated offline on the fixed dataset
  (e1 rel err 1.6e-4, e2 2.7e-7, total 4.2e-5).
"""

import numpy as np

N, D = 8192, 128
NCORES = 8
ROWS = N // NCORES          # 1024 rows per core
NRT = ROWS // 128           # 8 row-tiles per core
CH = 1024                   # PSUM chunk width (2 banks)
NCH = N // CH               # 8 chunks per row-tile
T_THR = 0.0025
MARGIN = 0.5
EPS = 1e-12
C0 = 0.5 + 0.25e-12         # dis = sqrt(s*(-0.5) + C0)
PACK_A = 4096.0             # c = A*s - t  (power of two: exact in bf16)
KNOCK_S = 16.0              # diagonal knock on s (diag -> ~-15)
KNOCK_C = PACK_A * KNOCK_S  # diagonal knock on c (knocks cancel in t_ii)
NEG = -1.0e30               # match_replace fill

_CACHE = {}


def _build_module():
    import os
    import concourse.bass as bass  # noqa: F401
    import concourse.tile as tile
    from contextlib import ExitStack
    from concourse import bacc, mybir

    STAGE = int(os.environ.get("BLCD_STAGE", "5"))
    import os as _os

    f32 = mybir.dt.float32
    bf16 = mybir.dt.bfloat16
    AF = mybir.ActivationFunctionType
    ALU = mybir.AluOpType
    AX = mybir.AxisListType

    nc = bacc.Bacc("TRN2", target_bir_lowering=False, debug=False,
                   num_devices=NCORES)

    yi_d = nc.dram_tensor("yi_rot", [N, D], f32, kind="ExternalInput")
    yit_d = nc.dram_tensor("yit_loc", [ROWS, D], f32, kind="ExternalInput")
    eye_d = nc.dram_tensor("eye1", [128, 128], f32, kind="ExternalInput")
    eyek_d = nc.dram_tensor("eyek", [128, 128], f32, kind="ExternalInput")
    eyekc_d = nc.dram_tensor("eyekc", [128, 128], f32, kind="ExternalInput")
    out_d = nc.dram_tensor("out", [128, 2 * NRT], f32, kind="ExternalOutput")
    DBG = os.environ.get("BLCD_DBG") == "1"
    if DBG:
        dbg_sk = nc.dram_tensor("dbg_sk", [128, 128], f32, kind="ExternalOutput")
        dbg_ck = nc.dram_tensor("dbg_ck", [128, 128], f32, kind="ExternalOutput")
        dbg_cs = nc.dram_tensor("dbg_cs", [128, 64], f32, kind="ExternalOutput")
        dbg_cc = nc.dram_tensor("dbg_cc", [128, 64], f32, kind="ExternalOutput")
        dbg_ps = nc.dram_tensor("dbg_ps", [128, 1024], f32, kind="ExternalOutput")
        dbg_pc = nc.dram_tensor("dbg_pc", [128, 1024], f32, kind="ExternalOutput")

    yi_r = yi_d.ap().rearrange("(n p) d -> p n d", p=128)     # [128, 64, 128]
    yit_r = yit_d.ap().rearrange("(n p) d -> p n d", p=128)   # [128, 8, 128]

    with tile.TileContext(nc) as tc, ExitStack() as ctx:
        cpool = ctx.enter_context(tc.tile_pool(name="consts", bufs=1))
        ppool = ctx.enter_context(tc.tile_pool(name="persist", bufs=1))
        smpool = ctx.enter_context(tc.tile_pool(name="small", bufs=4))

        eye = cpool.tile([128, 128], f32)
        eyek = cpool.tile([128, 128], f32)
        eyekc = cpool.tile([128, 128], f32)
        eyeb = cpool.tile([128, 128], bf16)
        knkb_s = cpool.tile([128, 128], bf16)
        knkb_c = cpool.tile([128, 128], bf16)
        c0b = cpool.tile([128, 1], f32)
        nc.gpsimd.memset(c0b[:], C0)
        epsb = cpool.tile([128, 1], f32)
        nc.gpsimd.memset(epsb[:], EPS)
        epsqb = cpool.tile([128, 1], f32)
        nc.gpsimd.memset(epsqb[:], EPS / 4.0)

        yinT = ppool.tile([128, N], bf16)       # normalized yi, transposed
        yinTA = ppool.tile([128, ROWS], bf16)   # 4096 * yinT local block
        yitTn = ppool.tile([128, ROWS], bf16)   # -normalized yi_t, transposed
        eacc = ppool.tile([128, 2 * NRT], f32)   # e1 cols 0:8, e2 cols 8:16
        nc.gpsimd.memset(eacc[:], 0.0)
        dis_td = ppool.tile([128, NRT], f32)    # dis(yin_i, yit_i) per tile

        n_rt = NRT if STAGE >= 5 else int(os.environ.get("BLCD_NRT", "1"))
        with tc.tile_pool(name="headbig", bufs=4) as hbig, \
             tc.tile_pool(name="headrows", bufs=10) as hrows, \
             tc.tile_pool(name="headkeep", bufs=1) as hkeep, \
             tc.tile_pool(name="headsm", bufs=4) as hsm, \
             tc.tile_pool(name="cands", bufs=2) as candp, \
             tc.tile_pool(name="ps_s", bufs=2, space="PSUM") as ps_spool, \
             tc.tile_pool(name="ps_c", bufs=2, space="PSUM") as ps_cpool:

            def fetch_rows(src_r, g):
                rows = hrows.tile([128, 8, 128], f32, tag="rows")
                nc.sync.dma_start(rows[:], src_r[:, g:g + 8, :])
                return rows

            def emit_head_group(rows, g, dsts, keep=False,
                                evict_dve=False):
                """dsts: list of (dstT, scale_mode); scale_mode in
                {'pos','neg','4k'} applied via the per-row rinv variant.
                Returns the rows_n bf16 tile of the last dst."""
                sq = hsm.tile([128, 8], f32, tag="sq")
                sqscr = hbig.tile([128, 8, 128], f32, tag="sqscr")
                nc.scalar.activation(
                    sqscr[:].rearrange("p a b -> p (a b)"),
                    rows[:].rearrange("p a b -> p (a b)"), AF.Square)
                nc.vector.tensor_reduce(sq[:], sqscr[:], op=ALU.add,
                                        axis=AX.X)
                nrm = hsm.tile([128, 8], f32, tag="nrm")
                nc.scalar.activation(nrm[:], sq[:], AF.Sqrt, bias=epsb[:])
                rinv = hsm.tile([128, 8], f32, tag="rinv")
                nc.vector.reciprocal(rinv[:], nrm[:])
                for (dstT, mode) in dsts:
                    if mode == 'pos':
                        rv = rinv
                    else:
                        rv = hsm.tile([128, 8], f32, tag="rv" + mode)
                        scl = -1.0 if mode == 'neg' else PACK_A
                        nc.vector.tensor_scalar(rv[:], rinv[:], scl, None,
                                                ALU.mult)
                    pool_n = hkeep if keep else hbig
                    rows_n = pool_n.tile([128, 8, 128], bf16,
                                         tag="rows_n" + mode)
                    for jj in range(8):
                        nc.gpsimd.tensor_scalar(rows_n[:, jj, :],
                                                rows[:, jj, :],
                                                rv[:, jj:jj + 1], None,
                                                ALU.mult)
                    ps = ps_spool.tile([128, CH], f32, tag="ps_s")
                    for jj in range(8):
                        nc.tensor.matmul(ps[:, jj * 128:(jj + 1) * 128],
                                         rows_n[:, jj, :], eyeb[:],
                                         start=True, stop=True)
                    if evict_dve:
                        nc.vector.tensor_copy(
                            dstT[:, g * 128:g * 128 + CH], ps[:])
                    else:
                        nc.scalar.copy(dstT[:, g * 128:g * 128 + CH], ps[:])
                return rows_n

            def emit_tail(rt, s_k, c_k, dve_only=False):
                # tail: recover t_k, hinge (deferred one tile for overlap;
                # small SBUF-only arithmetic runs on the idle Pool engine)
                t_k = smpool.tile([128, 16], f32, tag="tk")
                nc.vector.scalar_tensor_tensor(t_k[:], s_k[:], PACK_A,
                                               c_k[:], ALU.mult, ALU.subtract)
                dis_a = smpool.tile([128, 16], f32, tag="da")
                nc.scalar.activation(dis_a[:], s_k[:], AF.Sqrt,
                                     scale=-0.5, bias=c0b[:])
                dis_b = smpool.tile([128, 16], f32, tag="db")
                nc.scalar.activation(dis_b[:], t_k[:], AF.Sqrt,
                                     scale=-0.5, bias=c0b[:])
                eng = nc.vector if dve_only else nc.gpsimd
                diff = smpool.tile([128, 16], f32, tag="df")
                eng.tensor_sub(diff[:], dis_a[:], dis_b[:])
                sqd = smpool.tile([128, 16], f32, tag="sqd")
                eng.tensor_mul(sqd[:], diff[:], diff[:])
                hng = smpool.tile([128, 16], f32, tag="hg")
                eng.tensor_scalar(hng[:], sqd[:], T_THR, 0.0,
                                  ALU.subtract, ALU.max)
                hs2 = smpool.tile([128, 16], f32, tag="hs2")
                nc.vector.tensor_scalar(hs2[:], hng[:], 1.0, None,
                                        ALU.mult, ALU.add,
                                        accum_out=eacc[:, rt:rt + 1])
                # e2: dis_td + M - dis_nn, relu
                o2 = smpool.tile([128, 1], f32, tag="o2")
                nc.vector.scalar_tensor_tensor(o2[:], dis_a[:, 0:1], -1.0,
                                               dis_td[:, rt:rt + 1],
                                               ALU.mult, ALU.add)
                nc.vector.tensor_scalar(eacc[:, NRT + rt:NRT + rt + 1],
                                        o2[:], MARGIN, 0.0, ALU.add, ALU.max)

            # prefetch every row group before any compute is queued
            pre = [fetch_rows(yi_r, 0), fetch_rows(yit_r, 0)] + \
                  [fetch_rows(yi_r, g) for g in range(8, 64, 8)]
            nc.sync.dma_start(eye[:], eye_d[:])
            nc.sync.dma_start(eyek[:], eyek_d[:])
            nc.sync.dma_start(eyekc[:], eyekc_d[:])
            nc.gpsimd.tensor_copy(eyeb[:], eye[:])
            # knock matrices in bf16 (-16*eye, -65536*eye: exact in bf16)
            nc.gpsimd.tensor_scalar(knkb_s[:], eye[:], -KNOCK_S, None,
                                    ALU.mult)
            nc.gpsimd.tensor_scalar(knkb_c[:], eye[:], -KNOCK_C, None,
                                    ALU.mult)

            # group 0 covers the local block: also build the scaled lhs
            # copies (4096*yin and -yit, both exact bf16 transforms)
            rows_yi0 = emit_head_group(pre[0], 0, [(yinTA, '4k'),
                                                   (yinT, 'pos')],
                                       keep=True)
            rows_ytn = emit_head_group(pre[1], 0, [(yitTn, 'neg')],
                                       keep=True)


            def emit_chunk(rt, cc, cand_s, cand_c):
                lhs_s = yinT[:, rt * 128:(rt + 1) * 128]
                lhs_sA = yinTA[:, rt * 128:(rt + 1) * 128]
                lhs_tn = yitTn[:, rt * 128:(rt + 1) * 128]
                ps_s = ps_spool.tile([128, CH], f32)
                ps_c = ps_cpool.tile([128, CH], f32)
                # chunk 0 carries the self-block: split the matmuls at the
                # diagonal 128-column range and knock it with an extra
                # accumulated -K*eye matmul (lhsT=I => out += rhs); splits
                # never cross a 512 boundary (PSUM banks)
                if cc == 0:
                    splits = []
                    for h in range(2):
                        h0 = h * 512
                        d0 = rt * 128
                        if h0 <= d0 < h0 + 512:
                            splits += [(h0, d0 - h0, None),
                                       (d0, 128, 'knock'),
                                       (d0 + 128, h0 + 512 - d0 - 128, None)]
                        else:
                            splits.append((h0, 512, None))
                else:
                    splits = [(0, 512, None), (512, 512, None)]
                # all ps_s matmuls first so the s-scan starts early
                for (o, w, kn) in splits:
                    if w <= 0:
                        continue
                    hs = slice(o, o + w)
                    rhs = yinT[:, cc * CH + o: cc * CH + o + w]
                    nc.tensor.matmul(ps_s[:, hs], lhs_s, rhs,
                                     start=True, stop=kn is None)
                    if kn:
                        nc.tensor.matmul(ps_s[:, hs], eyeb[:], knkb_s[:],
                                         start=False, stop=True)
                for (o, w, kn) in splits:
                    if w <= 0:
                        continue
                    hs = slice(o, o + w)
                    rhs = yinT[:, cc * CH + o: cc * CH + o + w]
                    nc.tensor.matmul(ps_c[:, hs], lhs_sA, rhs,
                                     start=True, stop=False)
                    nc.tensor.matmul(ps_c[:, hs], lhs_tn, rhs,
                                     start=False, stop=kn is None)
                    if kn:
                        nc.tensor.matmul(ps_c[:, hs], eyeb[:], knkb_c[:],
                                         start=False, stop=True)
                # per-chunk top-8 candidates
                nc.vector.max(cand_s[:, cc * 8:(cc + 1) * 8], ps_s[:])
                nc.vector.max(cand_c[:, cc * 8:(cc + 1) * 8], ps_c[:])

            def emit_chains(rt, cand_s, cand_c):
                s_k = smpool.tile([128, 16], f32, tag="sk")
                c_k = smpool.tile([128, 16], f32, tag="ck")
                nc.vector.max(s_k[:, 0:8], cand_s[:])
                nc.vector.match_replace(cand_s[:], s_k[:, 0:8], cand_s[:],
                                        NEG)
                nc.vector.max(s_k[:, 8:16], cand_s[:])
                nc.vector.max(c_k[:, 0:8], cand_c[:])
                nc.vector.match_replace(cand_c[:], c_k[:, 0:8], cand_c[:],
                                        NEG)
                nc.vector.max(c_k[:, 8:16], cand_c[:])
                if DBG:
                    nc.sync.dma_start(dbg_sk[:, rt * 16:(rt + 1) * 16],
                                      s_k[:])
                    nc.sync.dma_start(dbg_ck[:, rt * 16:(rt + 1) * 16],
                                      c_k[:])
                return s_k, c_k

            def emit_dis_td():
                # dis(yin_i, yit_i) per local row, from normalized rows:
                # u = yin - yit; dis_td = sqrt(0.25*|u|^2 + eps/4)
                sqtd = smpool.tile([128, NRT], f32, tag="sqtd")
                for jj in range(NRT):
                    u_td = hbig.tile([128, 128], bf16, tag="u_td")
                    nc.gpsimd.tensor_add(u_td[:], rows_yi0[:, jj, :],
                                         rows_ytn[:, jj, :])
                    uscr = hbig.tile([128, 128], f32, tag="uscr")
                    nc.scalar.activation(uscr[:], u_td[:], AF.Square,
                                         accum_out=sqtd[:, jj:jj + 1])
                nc.scalar.activation(dis_td[:], sqtd[:], AF.Sqrt,
                                     scale=0.25, bias=epsqb[:])

            pending = None
            if n_rt >= 2:
                # tiles 0 and 1 run their chunk loops interleaved while the
                # head groups stream in: tile 1 scans earlier chunks while
                # tile 0 waits for later head groups, keeping the DVE busy
                cands01 = []
                for rt in (0, 1):
                    cand_s = candp.tile([128, NCH * 8], f32, tag="cs")
                    cand_c = candp.tile([128, NCH * 8], f32, tag="cc")
                    cands01.append((cand_s, cand_c))
                for cc in range(NCH):
                    if cc >= 1:
                        emit_head_group(pre[cc + 1], cc * 8,
                                        [(yinT, 'pos')])
                    emit_chunk(0, cc, *cands01[0])
                    emit_chunk(1, cc, *cands01[1])
                emit_dis_td()
                for rt in (0, 1):
                    s_k, c_k = emit_chains(rt, *cands01[rt])
                    if pending is not None:
                        emit_tail(*pending)
                    pending = (rt, s_k, c_k)
                first_rest = 2
            else:
                for cc in range(NCH):
                    if cc >= 1:
                        emit_head_group(pre[cc + 1], cc * 8,
                                        [(yinT, 'pos')])
                cand_s = candp.tile([128, NCH * 8], f32, tag="cs")
                cand_c = candp.tile([128, NCH * 8], f32, tag="cc")
                for cc in range(NCH):
                    emit_chunk(0, cc, cand_s, cand_c)
                emit_dis_td()
                s_k, c_k = emit_chains(0, cand_s, cand_c)
                pending = (0, s_k, c_k)
                first_rest = 1

            for rt in range(first_rest, n_rt):
                cand_s = candp.tile([128, NCH * 8], f32, tag="cs")
                cand_c = candp.tile([128, NCH * 8], f32, tag="cc")
                for cc in range(NCH):
                    emit_chunk(rt, cc, cand_s, cand_c)
                s_k, c_k = emit_chains(rt, cand_s, cand_c)
                if pending is not None:
                    emit_tail(*pending)
                pending = (rt, s_k, c_k)
            if pending is not None:
                emit_tail(*pending, dve_only=True)

        # ---------------- tail: one DMA of all partials (host sums) ------
        nc.sync.dma_start(out_d[:], eacc[:])

    nc.compile()
    return nc


def kernel(yi: np.ndarray, yi_t: np.ndarray):
    from concourse.bass_utils import run_bass_kernel_spmd

    if "nc" not in _CACHE:
        _CACHE["nc"] = _build_module()
    nc = _CACHE["nc"]

    yi = np.ascontiguousarray(np.asarray(yi, dtype=np.float32))
    yi_t = np.ascontiguousarray(np.asarray(yi_t, dtype=np.float32))
    eye1 = np.eye(128, dtype=np.float32)
    eyek = (KNOCK_S * np.eye(128)).astype(np.float32)
    eyekc = (KNOCK_C * np.eye(128)).astype(np.float32)

    in_maps = []
    for c in range(NCORES):
        lo = c * ROWS
        yi_rot = np.concatenate([yi[lo:], yi[:lo]], axis=0)
        in_maps.append({
            "yi_rot": np.ascontiguousarray(yi_rot),
            "yit_loc": np.ascontiguousarray(yi_t[lo:lo + ROWS]),
            "eye1": eye1,
            "eyek": eyek,
            "eyekc": eyekc,
        })

    res = run_bass_kernel_spmd(nc, in_maps, list(range(NCORES))).results

    e1 = np.float64(0.0)
    e2 = np.float64(0.0)
    for c in range(NCORES):
        out = res[c]["out"]
        e1 += out[:, 0:NRT].astype(np.float64).sum()
        e2 += out[:, NRT:2 * NRT].astype(np.float64).sum()
    e1 = np.float32(e1)
    e2 = np.float32(e2)
    return (np.float32(e1 + e2), e1, e2)


# revision 16
# speedup vs baseline: 1.0369x; 1.0069x over previous
"""Trainium2 Bass kernel for nn_BLCD_Loss (retrieval kNN hinge loss) — v3.

Math (reference):
  yin = l2norm(yi), yit = l2norm(yi_t)
  top-16 neighbors of each yin_i among all yin_j (by cosine sim s = yin yinT)
  e1 = sum_k relu((dis(yin_i,yj_k) - dis(yit_i,yj_k))^2 - T)
  e2 = sum relu(dis(yin_i,yit_i) + M - dis(yin_i,yj_0))

Kernel strategy (8 cores, SPMD), "PACK3":
  Each core owns 1024 rows (host rotates yi so the self-diagonal block is in
  column chunk 0 on every core).  Per 128-row tile and 1024-column chunk the
  PE computes TWO bf16 matmuls into PSUM:
    s = yin_loc @ yinT
    c = 4096*s - t   (accumulated: lhs 4096*yin_loc then lhs -yit_loc;
                      4096*x and -x are exact in bf16, so c is an exact
                      function of the same hardware products as s and t)
  The DVE runs two top-16 value chains (per-1024-chunk max8 -> 64 candidates
  -> max8 + match_replace + max8): one on s, one on c.  Both rank almost
  identically (t/4096 perturbation), so rank-k pairs recover the neighbor
  t-similarity exactly: t_k = 4096*s_k - c_k.  The hinge then runs on tiny
  [128,16] arrays - no full-row sqrt, mask, hinge, or gather passes at all.
  Head: normalize rows (ACT square+Sqrt, DVE reduce+recip, Pool scale to
  bf16), transpose via bf16 identity matmul on the PE, sharing the ps_s PSUM
  ring; head groups 1..7 stream inside tile-0's chunk loop just ahead of
  the chunks that consume them.
  Selection fidelity & rank-pairing validated offline on the fixed dataset
  (e1 rel err 1.6e-4, e2 2.7e-7, total 4.2e-5).
"""

import numpy as np

N, D = 8192, 128
NCORES = 8
ROWS = N // NCORES          # 1024 rows per core
NRT = ROWS // 128           # 8 row-tiles per core
CH = 1024                   # PSUM chunk width (2 banks)
NCH = N // CH               # 8 chunks per row-tile
T_THR = 0.0025
MARGIN = 0.5
EPS = 1e-12
C0 = 0.5 + 0.25e-12         # dis = sqrt(s*(-0.5) + C0)
PACK_A = 4096.0             # c = A*s - t  (power of two: exact in bf16)
KNOCK_S = 16.0              # diagonal knock on s (diag -> ~-15)
KNOCK_C = PACK_A * KNOCK_S  # diagonal knock on c (knocks cancel in t_ii)
NEG = -1.0e30               # match_replace fill

_CACHE = {}


def _build_module():
    import os
    import concourse.bass as bass  # noqa: F401
    import concourse.tile as tile
    from contextlib import ExitStack
    from concourse import bacc, mybir

    STAGE = int(os.environ.get("BLCD_STAGE", "5"))
    import os as _os

    f32 = mybir.dt.float32
    bf16 = mybir.dt.bfloat16
    AF = mybir.ActivationFunctionType
    ALU = mybir.AluOpType
    AX = mybir.AxisListType

    nc = bacc.Bacc("TRN2", target_bir_lowering=False, debug=False,
                   num_devices=NCORES)

    yi_d = nc.dram_tensor("yi_rot", [N, D], f32, kind="ExternalInput")
    yit_d = nc.dram_tensor("yit_loc", [ROWS, D], f32, kind="ExternalInput")
    eye_d = nc.dram_tensor("eye1", [128, 128], f32, kind="ExternalInput")
    eyek_d = nc.dram_tensor("eyek", [128, 128], f32, kind="ExternalInput")
    eyekc_d = nc.dram_tensor("eyekc", [128, 128], f32, kind="ExternalInput")
    out_d = nc.dram_tensor("out", [128, 2 * NRT], f32, kind="ExternalOutput")
    DBG = os.environ.get("BLCD_DBG") == "1"
    if DBG:
        dbg_sk = nc.dram_tensor("dbg_sk", [128, 128], f32, kind="ExternalOutput")
        dbg_ck = nc.dram_tensor("dbg_ck", [128, 128], f32, kind="ExternalOutput")
        dbg_cs = nc.dram_tensor("dbg_cs", [128, 64], f32, kind="ExternalOutput")
        dbg_cc = nc.dram_tensor("dbg_cc", [128, 64], f32, kind="ExternalOutput")
        dbg_ps = nc.dram_tensor("dbg_ps", [128, 1024], f32, kind="ExternalOutput")
        dbg_pc = nc.dram_tensor("dbg_pc", [128, 1024], f32, kind="ExternalOutput")

    yi_r = yi_d.ap().rearrange("(n p) d -> p n d", p=128)     # [128, 64, 128]
    yit_r = yit_d.ap().rearrange("(n p) d -> p n d", p=128)   # [128, 8, 128]

    with tile.TileContext(nc) as tc, ExitStack() as ctx:
        cpool = ctx.enter_context(tc.tile_pool(name="consts", bufs=1))
        ppool = ctx.enter_context(tc.tile_pool(name="persist", bufs=1))
        smpool = ctx.enter_context(tc.tile_pool(name="small", bufs=4))

        eye = cpool.tile([128, 128], f32)
        eyek = cpool.tile([128, 128], f32)
        eyekc = cpool.tile([128, 128], f32)
        eyeb = cpool.tile([128, 128], bf16)
        knkb_s = cpool.tile([128, 128], bf16)
        knkb_c = cpool.tile([128, 128], bf16)
        c0b = cpool.tile([128, 1], f32)
        nc.gpsimd.memset(c0b[:], C0)
        epsb = cpool.tile([128, 1], f32)
        nc.gpsimd.memset(epsb[:], EPS)
        epsqb = cpool.tile([128, 1], f32)
        nc.gpsimd.memset(epsqb[:], EPS / 4.0)

        yinT = ppool.tile([128, N], bf16)       # normalized yi, transposed
        yinTA = ppool.tile([128, ROWS], bf16)   # 4096 * yinT local block
        yitTn = ppool.tile([128, ROWS], bf16)   # -normalized yi_t, transposed
        eacc = ppool.tile([128, 2 * NRT], f32)   # e1 cols 0:8, e2 cols 8:16
        nc.gpsimd.memset(eacc[:], 0.0)
        dis_td = ppool.tile([128, NRT], f32)    # dis(yin_i, yit_i) per tile

        n_rt = NRT if STAGE >= 5 else int(os.environ.get("BLCD_NRT", "1"))
        with tc.tile_pool(name="headbig", bufs=4) as hbig, \
             tc.tile_pool(name="headrows", bufs=10) as hrows, \
             tc.tile_pool(name="headkeep", bufs=1) as hkeep, \
             tc.tile_pool(name="headsm", bufs=4) as hsm, \
             tc.tile_pool(name="cands", bufs=2) as candp, \
             tc.tile_pool(name="ps_s", bufs=2, space="PSUM") as ps_spool, \
             tc.tile_pool(name="ps_c", bufs=2, space="PSUM") as ps_cpool:

            def fetch_rows(src_r, g, split=1):
                rows = hrows.tile([128, 8, 128], f32, tag="rows")
                hb = 8 // split
                for q in range(split):
                    nc.sync.dma_start(rows[:, q * hb:(q + 1) * hb, :],
                                      src_r[:, g + q * hb:g + (q + 1) * hb,
                                            :])
                return rows

            def emit_head_group(rows, g, dsts, keep=False,
                                evict_dve=False, split=1):
                """dsts: list of (dstT, scale_mode); scale_mode in
                {'pos','neg','4k'} applied via the per-row rinv variant.
                Returns the rows_n bf16 tile of the last dst."""
                sq = hsm.tile([128, 8], f32, tag="sq")
                sqscr = hbig.tile([128, 8, 128], f32, tag="sqscr")
                hb = 8 // split
                for q in range(split):
                    qs = slice(q * hb, (q + 1) * hb)
                    nc.scalar.activation(
                        sqscr[:, qs, :].rearrange("p a b -> p (a b)"),
                        rows[:, qs, :].rearrange("p a b -> p (a b)"),
                        AF.Square)
                    nc.vector.tensor_reduce(sq[:, qs], sqscr[:, qs, :],
                                            op=ALU.add, axis=AX.X)
                nrm = hsm.tile([128, 8], f32, tag="nrm")
                nc.scalar.activation(nrm[:], sq[:], AF.Sqrt, bias=epsb[:])
                rinv = hsm.tile([128, 8], f32, tag="rinv")
                nc.vector.reciprocal(rinv[:], nrm[:])
                for (dstT, mode) in dsts:
                    if mode == 'pos':
                        rv = rinv
                    else:
                        rv = hsm.tile([128, 8], f32, tag="rv" + mode)
                        scl = -1.0 if mode == 'neg' else PACK_A
                        nc.vector.tensor_scalar(rv[:], rinv[:], scl, None,
                                                ALU.mult)
                    pool_n = hkeep if keep else hbig
                    rows_n = pool_n.tile([128, 8, 128], bf16,
                                         tag="rows_n" + mode)
                    for jj in range(8):
                        nc.gpsimd.tensor_scalar(rows_n[:, jj, :],
                                                rows[:, jj, :],
                                                rv[:, jj:jj + 1], None,
                                                ALU.mult)
                    ps = ps_spool.tile([128, CH], f32, tag="ps_s")
                    for jj in range(8):
                        nc.tensor.matmul(ps[:, jj * 128:(jj + 1) * 128],
                                         rows_n[:, jj, :], eyeb[:],
                                         start=True, stop=True)
                    if evict_dve:
                        nc.vector.tensor_copy(
                            dstT[:, g * 128:g * 128 + CH], ps[:])
                    else:
                        nc.scalar.copy(dstT[:, g * 128:g * 128 + CH], ps[:])
                return rows_n

            def emit_tail(rt, s_k, c_k, dve_only=False):
                # tail: recover t_k, hinge (deferred one tile for overlap;
                # small SBUF-only arithmetic runs on the idle Pool engine)
                t_k = smpool.tile([128, 16], f32, tag="tk")
                nc.vector.scalar_tensor_tensor(t_k[:], s_k[:], PACK_A,
                                               c_k[:], ALU.mult, ALU.subtract)
                dis_a = smpool.tile([128, 16], f32, tag="da")
                nc.scalar.activation(dis_a[:], s_k[:], AF.Sqrt,
                                     scale=-0.5, bias=c0b[:])
                dis_b = smpool.tile([128, 16], f32, tag="db")
                nc.scalar.activation(dis_b[:], t_k[:], AF.Sqrt,
                                     scale=-0.5, bias=c0b[:])
                eng = nc.vector if dve_only else nc.gpsimd
                diff = smpool.tile([128, 16], f32, tag="df")
                eng.tensor_sub(diff[:], dis_a[:], dis_b[:])
                sqd = smpool.tile([128, 16], f32, tag="sqd")
                eng.tensor_mul(sqd[:], diff[:], diff[:])
                hng = smpool.tile([128, 16], f32, tag="hg")
                eng.tensor_scalar(hng[:], sqd[:], T_THR, 0.0,
                                  ALU.subtract, ALU.max)
                hs2 = smpool.tile([128, 16], f32, tag="hs2")
                nc.vector.tensor_scalar(hs2[:], hng[:], 1.0, None,
                                        ALU.mult, ALU.add,
                                        accum_out=eacc[:, rt:rt + 1])
                # e2: dis_td + M - dis_nn, relu
                o2 = smpool.tile([128, 1], f32, tag="o2")
                nc.vector.scalar_tensor_tensor(o2[:], dis_a[:, 0:1], -1.0,
                                               dis_td[:, rt:rt + 1],
                                               ALU.mult, ALU.add)
                nc.vector.tensor_scalar(eacc[:, NRT + rt:NRT + rt + 1],
                                        o2[:], MARGIN, 0.0, ALU.add, ALU.max)

            # prefetch every row group before any compute is queued
            pre = [fetch_rows(yi_r, 0, split=2),
                   fetch_rows(yit_r, 0, split=2)] + \
                  [fetch_rows(yi_r, g) for g in range(8, 64, 8)]
            nc.sync.dma_start(eye[:], eye_d[:])
            nc.sync.dma_start(eyek[:], eyek_d[:])
            nc.sync.dma_start(eyekc[:], eyekc_d[:])
            nc.gpsimd.tensor_copy(eyeb[:], eye[:])
            # knock matrices in bf16 (-16*eye, -65536*eye: exact in bf16)
            nc.gpsimd.tensor_scalar(knkb_s[:], eye[:], -KNOCK_S, None,
                                    ALU.mult)
            nc.gpsimd.tensor_scalar(knkb_c[:], eye[:], -KNOCK_C, None,
                                    ALU.mult)

            # group 0 covers the local block: also build the scaled lhs
            # copies (4096*yin and -yit, both exact bf16 transforms)
            rows_yi0 = emit_head_group(pre[0], 0, [(yinTA, '4k'),
                                                   (yinT, 'pos')],
                                       keep=True, split=2)
            rows_ytn = emit_head_group(pre[1], 0, [(yitTn, 'neg')],
                                       keep=True, split=2)


            def emit_chunk(rt, cc, cand_s, cand_c):
                lhs_s = yinT[:, rt * 128:(rt + 1) * 128]
                lhs_sA = yinTA[:, rt * 128:(rt + 1) * 128]
                lhs_tn = yitTn[:, rt * 128:(rt + 1) * 128]
                ps_s = ps_spool.tile([128, CH], f32)
                ps_c = ps_cpool.tile([128, CH], f32)
                # chunk 0 carries the self-block: split the matmuls at the
                # diagonal 128-column range and knock it with an extra
                # accumulated -K*eye matmul (lhsT=I => out += rhs); splits
                # never cross a 512 boundary (PSUM banks)
                if cc == 0:
                    splits = []
                    for h in range(2):
                        h0 = h * 512
                        d0 = rt * 128
                        if h0 <= d0 < h0 + 512:
                            splits += [(h0, d0 - h0, None),
                                       (d0, 128, 'knock'),
                                       (d0 + 128, h0 + 512 - d0 - 128, None)]
                        else:
                            splits.append((h0, 512, None))
                else:
                    splits = [(0, 512, None), (512, 512, None)]
                # all ps_s matmuls first so the s-scan starts early
                for (o, w, kn) in splits:
                    if w <= 0:
                        continue
                    hs = slice(o, o + w)
                    rhs = yinT[:, cc * CH + o: cc * CH + o + w]
                    nc.tensor.matmul(ps_s[:, hs], lhs_s, rhs,
                                     start=True, stop=kn is None)
                    if kn:
                        nc.tensor.matmul(ps_s[:, hs], eyeb[:], knkb_s[:],
                                         start=False, stop=True)
                for (o, w, kn) in splits:
                    if w <= 0:
                        continue
                    hs = slice(o, o + w)
                    rhs = yinT[:, cc * CH + o: cc * CH + o + w]
                    nc.tensor.matmul(ps_c[:, hs], lhs_sA, rhs,
                                     start=True, stop=False)
                    nc.tensor.matmul(ps_c[:, hs], lhs_tn, rhs,
                                     start=False, stop=kn is None)
                    if kn:
                        nc.tensor.matmul(ps_c[:, hs], eyeb[:], knkb_c[:],
                                         start=False, stop=True)
                # per-chunk top-8 candidates
                nc.vector.max(cand_s[:, cc * 8:(cc + 1) * 8], ps_s[:])
                nc.vector.max(cand_c[:, cc * 8:(cc + 1) * 8], ps_c[:])

            def emit_chains(rt, cand_s, cand_c):
                s_k = smpool.tile([128, 16], f32, tag="sk")
                c_k = smpool.tile([128, 16], f32, tag="ck")
                nc.vector.max(s_k[:, 0:8], cand_s[:])
                nc.vector.match_replace(cand_s[:], s_k[:, 0:8], cand_s[:],
                                        NEG)
                nc.vector.max(s_k[:, 8:16], cand_s[:])
                nc.vector.max(c_k[:, 0:8], cand_c[:])
                nc.vector.match_replace(cand_c[:], c_k[:, 0:8], cand_c[:],
                                        NEG)
                nc.vector.max(c_k[:, 8:16], cand_c[:])
                if DBG:
                    nc.sync.dma_start(dbg_sk[:, rt * 16:(rt + 1) * 16],
                                      s_k[:])
                    nc.sync.dma_start(dbg_ck[:, rt * 16:(rt + 1) * 16],
                                      c_k[:])
                return s_k, c_k

            def emit_dis_td():
                # dis(yin_i, yit_i) per local row, from normalized rows:
                # u = yin - yit; dis_td = sqrt(0.25*|u|^2 + eps/4)
                sqtd = smpool.tile([128, NRT], f32, tag="sqtd")
                for jj in range(NRT):
                    u_td = hbig.tile([128, 128], bf16, tag="u_td")
                    nc.gpsimd.tensor_add(u_td[:], rows_yi0[:, jj, :],
                                         rows_ytn[:, jj, :])
                    uscr = hbig.tile([128, 128], f32, tag="uscr")
                    nc.scalar.activation(uscr[:], u_td[:], AF.Square,
                                         accum_out=sqtd[:, jj:jj + 1])
                nc.scalar.activation(dis_td[:], sqtd[:], AF.Sqrt,
                                     scale=0.25, bias=epsqb[:])

            pending = None
            if n_rt >= 2:
                # tiles 0 and 1 run their chunk loops interleaved while the
                # head groups stream in: tile 1 scans earlier chunks while
                # tile 0 waits for later head groups, keeping the DVE busy
                cands01 = []
                for rt in (0, 1):
                    cand_s = candp.tile([128, NCH * 8], f32, tag="cs")
                    cand_c = candp.tile([128, NCH * 8], f32, tag="cc")
                    cands01.append((cand_s, cand_c))
                for cc in range(NCH):
                    if cc >= 1:
                        emit_head_group(pre[cc + 1], cc * 8,
                                        [(yinT, 'pos')])
                    emit_chunk(0, cc, *cands01[0])
                    emit_chunk(1, cc, *cands01[1])
                emit_dis_td()
                for rt in (0, 1):
                    s_k, c_k = emit_chains(rt, *cands01[rt])
                    if pending is not None:
                        emit_tail(*pending)
                    pending = (rt, s_k, c_k)
                first_rest = 2
            else:
                for cc in range(NCH):
                    if cc >= 1:
                        emit_head_group(pre[cc + 1], cc * 8,
                                        [(yinT, 'pos')])
                cand_s = candp.tile([128, NCH * 8], f32, tag="cs")
                cand_c = candp.tile([128, NCH * 8], f32, tag="cc")
                for cc in range(NCH):
                    emit_chunk(0, cc, cand_s, cand_c)
                emit_dis_td()
                s_k, c_k = emit_chains(0, cand_s, cand_c)
                pending = (0, s_k, c_k)
                first_rest = 1

            for rt in range(first_rest, n_rt):
                cand_s = candp.tile([128, NCH * 8], f32, tag="cs")
                cand_c = candp.tile([128, NCH * 8], f32, tag="cc")
                for cc in range(NCH):
                    emit_chunk(rt, cc, cand_s, cand_c)
                s_k, c_k = emit_chains(rt, cand_s, cand_c)
                if pending is not None:
                    emit_tail(*pending)
                pending = (rt, s_k, c_k)
            if pending is not None:
                emit_tail(*pending, dve_only=True)

        # ---------------- tail: one DMA of all partials (host sums) ------
        nc.sync.dma_start(out_d[:], eacc[:])

    nc.compile()
    return nc


def kernel(yi: np.ndarray, yi_t: np.ndarray):
    from concourse.bass_utils import run_bass_kernel_spmd

    if "nc" not in _CACHE:
        _CACHE["nc"] = _build_module()
    nc = _CACHE["nc"]

    yi = np.ascontiguousarray(np.asarray(yi, dtype=np.float32))
    yi_t = np.ascontiguousarray(np.asarray(yi_t, dtype=np.float32))
    eye1 = np.eye(128, dtype=np.float32)
    eyek = (KNOCK_S * np.eye(128)).astype(np.float32)
    eyekc = (KNOCK_C * np.eye(128)).astype(np.float32)

    in_maps = []
    for c in range(NCORES):
        lo = c * ROWS
        yi_rot = np.concatenate([yi[lo:], yi[:lo]], axis=0)
        in_maps.append({
            "yi_rot": np.ascontiguousarray(yi_rot),
            "yit_loc": np.ascontiguousarray(yi_t[lo:lo + ROWS]),
            "eye1": eye1,
            "eyek": eyek,
            "eyekc": eyekc,
        })

    res = run_bass_kernel_spmd(nc, in_maps, list(range(NCORES))).results

    e1 = np.float64(0.0)
    e2 = np.float64(0.0)
    for c in range(NCORES):
        out = res[c]["out"]
        e1 += out[:, 0:NRT].astype(np.float64).sum()
        e2 += out[:, NRT:2 * NRT].astype(np.float64).sum()
    e1 = np.float32(e1)
    e2 = np.float32(e2)
    return (np.float32(e1 + e2), e1, e2)


# revision 17
# speedup vs baseline: 1.0403x; 1.0033x over previous
"""Trainium2 Bass kernel for nn_BLCD_Loss (retrieval kNN hinge loss) — v3.

Math (reference):
  yin = l2norm(yi), yit = l2norm(yi_t)
  top-16 neighbors of each yin_i among all yin_j (by cosine sim s = yin yinT)
  e1 = sum_k relu((dis(yin_i,yj_k) - dis(yit_i,yj_k))^2 - T)
  e2 = sum relu(dis(yin_i,yit_i) + M - dis(yin_i,yj_0))

Kernel strategy (8 cores, SPMD), "PACK3":
  Each core owns 1024 rows (host rotates yi so the self-diagonal block is in
  column chunk 0 on every core).  Per 128-row tile and 1024-column chunk the
  PE computes TWO bf16 matmuls into PSUM:
    s = yin_loc @ yinT
    c = 4096*s - t   (accumulated: lhs 4096*yin_loc then lhs -yit_loc;
                      4096*x and -x are exact in bf16, so c is an exact
                      function of the same hardware products as s and t)
  The DVE runs two top-16 value chains (per-1024-chunk max8 -> 64 candidates
  -> max8 + match_replace + max8): one on s, one on c.  Both rank almost
  identically (t/4096 perturbation), so rank-k pairs recover the neighbor
  t-similarity exactly: t_k = 4096*s_k - c_k.  The hinge then runs on tiny
  [128,16] arrays - no full-row sqrt, mask, hinge, or gather passes at all.
  Head: normalize rows (ACT square+Sqrt, DVE reduce+recip, Pool scale to
  bf16), transpose via bf16 identity matmul on the PE, sharing the ps_s PSUM
  ring; head groups 1..7 stream inside tile-0's chunk loop just ahead of
  the chunks that consume them.
  Selection fidelity & rank-pairing validated offline on the fixed dataset
  (e1 rel err 1.6e-4, e2 2.7e-7, total 4.2e-5).
"""

import numpy as np

N, D = 8192, 128
NCORES = 8
ROWS = N // NCORES          # 1024 rows per core
NRT = ROWS // 128           # 8 row-tiles per core
CH = 1024                   # PSUM chunk width (2 banks)
NCH = N // CH               # 8 chunks per row-tile
T_THR = 0.0025
MARGIN = 0.5
EPS = 1e-12
C0 = 0.5 + 0.25e-12         # dis = sqrt(s*(-0.5) + C0)
PACK_A = 4096.0             # c = A*s - t  (power of two: exact in bf16)
KNOCK_S = 16.0              # diagonal knock on s (diag -> ~-15)
KNOCK_C = PACK_A * KNOCK_S  # diagonal knock on c (knocks cancel in t_ii)
NEG = -1.0e30               # match_replace fill

_CACHE = {}


def _build_module():
    import os
    import concourse.bass as bass  # noqa: F401
    import concourse.tile as tile
    from contextlib import ExitStack
    from concourse import bacc, mybir

    STAGE = int(os.environ.get("BLCD_STAGE", "5"))
    import os as _os

    f32 = mybir.dt.float32
    bf16 = mybir.dt.bfloat16
    AF = mybir.ActivationFunctionType
    ALU = mybir.AluOpType
    AX = mybir.AxisListType

    nc = bacc.Bacc("TRN2", target_bir_lowering=False, debug=False,
                   num_devices=NCORES)

    yi_d = nc.dram_tensor("yi_rot", [N, D], f32, kind="ExternalInput")
    yit_d = nc.dram_tensor("yit_loc", [ROWS, D], f32, kind="ExternalInput")
    eye_d = nc.dram_tensor("eye1", [128, 128], f32, kind="ExternalInput")
    eyek_d = nc.dram_tensor("eyek", [128, 128], f32, kind="ExternalInput")
    eyekc_d = nc.dram_tensor("eyekc", [128, 128], f32, kind="ExternalInput")
    out_d = nc.dram_tensor("out", [128, 2 * NRT], f32, kind="ExternalOutput")
    DBG = os.environ.get("BLCD_DBG") == "1"
    if DBG:
        dbg_sk = nc.dram_tensor("dbg_sk", [128, 128], f32, kind="ExternalOutput")
        dbg_ck = nc.dram_tensor("dbg_ck", [128, 128], f32, kind="ExternalOutput")
        dbg_cs = nc.dram_tensor("dbg_cs", [128, 64], f32, kind="ExternalOutput")
        dbg_cc = nc.dram_tensor("dbg_cc", [128, 64], f32, kind="ExternalOutput")
        dbg_ps = nc.dram_tensor("dbg_ps", [128, 1024], f32, kind="ExternalOutput")
        dbg_pc = nc.dram_tensor("dbg_pc", [128, 1024], f32, kind="ExternalOutput")

    yi_r = yi_d.ap().rearrange("(n p) d -> p n d", p=128)     # [128, 64, 128]
    yit_r = yit_d.ap().rearrange("(n p) d -> p n d", p=128)   # [128, 8, 128]

    with tile.TileContext(nc) as tc, ExitStack() as ctx:
        cpool = ctx.enter_context(tc.tile_pool(name="consts", bufs=1))
        ppool = ctx.enter_context(tc.tile_pool(name="persist", bufs=1))
        smpool = ctx.enter_context(tc.tile_pool(name="small", bufs=4))

        eye = cpool.tile([128, 128], f32)
        eyek = cpool.tile([128, 128], f32)
        eyekc = cpool.tile([128, 128], f32)
        eyeb = cpool.tile([128, 128], bf16)
        knkb_s = cpool.tile([128, 128], bf16)
        knkb_c = cpool.tile([128, 128], bf16)
        c0b = cpool.tile([128, 1], f32)
        nc.gpsimd.memset(c0b[:], C0)
        epsb = cpool.tile([128, 1], f32)
        nc.gpsimd.memset(epsb[:], EPS)
        epsqb = cpool.tile([128, 1], f32)
        nc.gpsimd.memset(epsqb[:], EPS / 4.0)

        yinT = ppool.tile([128, N], bf16)       # normalized yi, transposed
        yinTA = ppool.tile([128, ROWS], bf16)   # 4096 * yinT local block
        yitTn = ppool.tile([128, ROWS], bf16)   # -normalized yi_t, transposed
        eacc = ppool.tile([128, 2 * NRT], f32)   # e1 cols 0:8, e2 cols 8:16
        nc.gpsimd.memset(eacc[:], 0.0)
        dis_td = ppool.tile([128, NRT], f32)    # dis(yin_i, yit_i) per tile

        n_rt = NRT if STAGE >= 5 else int(os.environ.get("BLCD_NRT", "1"))
        with tc.tile_pool(name="headbig", bufs=4) as hbig, \
             tc.tile_pool(name="headrows", bufs=10) as hrows, \
             tc.tile_pool(name="headkeep", bufs=1) as hkeep, \
             tc.tile_pool(name="headsm", bufs=4) as hsm, \
             tc.tile_pool(name="cands", bufs=2) as candp, \
             tc.tile_pool(name="ps_s", bufs=2, space="PSUM") as ps_spool, \
             tc.tile_pool(name="ps_c", bufs=2, space="PSUM") as ps_cpool:

            def fetch_rows(src_r, g, split=1):
                rows = hrows.tile([128, 8, 128], f32, tag="rows")
                hb = 8 // split
                for q in range(split):
                    nc.sync.dma_start(rows[:, q * hb:(q + 1) * hb, :],
                                      src_r[:, g + q * hb:g + (q + 1) * hb,
                                            :])
                return rows

            def emit_head_group(rows, g, dsts, keep=False,
                                evict_dve=False, split=1):
                """dsts: list of (dstT, scale_mode); scale_mode in
                {'pos','neg','4k'} applied via the per-row rinv variant.
                Returns the rows_n bf16 tile of the last dst."""
                sq = hsm.tile([128, 8], f32, tag="sq")
                sqscr = hbig.tile([128, 8, 128], f32, tag="sqscr")
                hb = 8 // split
                for q in range(split):
                    qs = slice(q * hb, (q + 1) * hb)
                    nc.scalar.activation(
                        sqscr[:, qs, :].rearrange("p a b -> p (a b)"),
                        rows[:, qs, :].rearrange("p a b -> p (a b)"),
                        AF.Square)
                    nc.vector.tensor_reduce(sq[:, qs], sqscr[:, qs, :],
                                            op=ALU.add, axis=AX.X)
                nrm = hsm.tile([128, 8], f32, tag="nrm")
                nc.scalar.activation(nrm[:], sq[:], AF.Sqrt, bias=epsb[:])
                rinv = hsm.tile([128, 8], f32, tag="rinv")
                nc.vector.reciprocal(rinv[:], nrm[:])
                for (dstT, mode) in dsts:
                    if mode == 'pos':
                        rv = rinv
                    else:
                        rv = hsm.tile([128, 8], f32, tag="rv" + mode)
                        scl = -1.0 if mode == 'neg' else PACK_A
                        nc.vector.tensor_scalar(rv[:], rinv[:], scl, None,
                                                ALU.mult)
                    pool_n = hkeep if keep else hbig
                    rows_n = pool_n.tile([128, 8, 128], bf16,
                                         tag="rows_n" + mode)
                    for jj in range(8):
                        nc.gpsimd.tensor_scalar(rows_n[:, jj, :],
                                                rows[:, jj, :],
                                                rv[:, jj:jj + 1], None,
                                                ALU.mult)
                    ps = ps_spool.tile([128, CH], f32, tag="ps_s")
                    for jj in range(8):
                        nc.tensor.matmul(ps[:, jj * 128:(jj + 1) * 128],
                                         rows_n[:, jj, :], eyeb[:],
                                         start=True, stop=True)
                    if evict_dve:
                        nc.vector.tensor_copy(
                            dstT[:, g * 128:g * 128 + CH], ps[:])
                    else:
                        nc.scalar.copy(dstT[:, g * 128:g * 128 + CH], ps[:])
                return rows_n

            def emit_tail(rt, s_k, c_k, dve_only=False):
                # tail: recover t_k, hinge (deferred one tile for overlap;
                # small SBUF-only arithmetic runs on the idle Pool engine)
                t_k = smpool.tile([128, 16], f32, tag="tk")
                nc.vector.scalar_tensor_tensor(t_k[:], s_k[:], PACK_A,
                                               c_k[:], ALU.mult, ALU.subtract)
                dis_a = smpool.tile([128, 16], f32, tag="da")
                nc.scalar.activation(dis_a[:], s_k[:], AF.Sqrt,
                                     scale=-0.5, bias=c0b[:])
                dis_b = smpool.tile([128, 16], f32, tag="db")
                nc.scalar.activation(dis_b[:], t_k[:], AF.Sqrt,
                                     scale=-0.5, bias=c0b[:])
                eng = nc.vector if dve_only else nc.gpsimd
                diff = smpool.tile([128, 16], f32, tag="df")
                eng.tensor_sub(diff[:], dis_a[:], dis_b[:])
                sqd = smpool.tile([128, 16], f32, tag="sqd")
                eng.tensor_mul(sqd[:], diff[:], diff[:])
                hng = smpool.tile([128, 16], f32, tag="hg")
                eng.tensor_scalar(hng[:], sqd[:], T_THR, 0.0,
                                  ALU.subtract, ALU.max)
                hs2 = smpool.tile([128, 16], f32, tag="hs2")
                nc.vector.tensor_scalar(hs2[:], hng[:], 1.0, None,
                                        ALU.mult, ALU.add,
                                        accum_out=eacc[:, rt:rt + 1])
                # e2: dis_td + M - dis_nn, relu
                o2 = smpool.tile([128, 1], f32, tag="o2")
                nc.vector.scalar_tensor_tensor(o2[:], dis_a[:, 0:1], -1.0,
                                               dis_td[:, rt:rt + 1],
                                               ALU.mult, ALU.add)
                nc.vector.tensor_scalar(eacc[:, NRT + rt:NRT + rt + 1],
                                        o2[:], MARGIN, 0.0, ALU.add, ALU.max)

            # prefetch every row group before any compute is queued
            pre = [fetch_rows(yi_r, 0, split=2),
                   fetch_rows(yit_r, 0, split=2)] + \
                  [fetch_rows(yi_r, g, split=2) for g in range(8, 64, 8)]
            nc.sync.dma_start(eye[:], eye_d[:])
            nc.sync.dma_start(eyek[:], eyek_d[:])
            nc.sync.dma_start(eyekc[:], eyekc_d[:])
            nc.gpsimd.tensor_copy(eyeb[:], eye[:])
            # knock matrices in bf16 (-16*eye, -65536*eye: exact in bf16)
            nc.gpsimd.tensor_scalar(knkb_s[:], eye[:], -KNOCK_S, None,
                                    ALU.mult)
            nc.gpsimd.tensor_scalar(knkb_c[:], eye[:], -KNOCK_C, None,
                                    ALU.mult)

            # group 0 covers the local block: also build the scaled lhs
            # copies (4096*yin and -yit, both exact bf16 transforms)
            rows_yi0 = emit_head_group(pre[0], 0, [(yinTA, '4k'),
                                                   (yinT, 'pos')],
                                       keep=True, split=2)
            rows_ytn = emit_head_group(pre[1], 0, [(yitTn, 'neg')],
                                       keep=True, split=2)


            def emit_chunk(rt, cc, cand_s, cand_c):
                lhs_s = yinT[:, rt * 128:(rt + 1) * 128]
                lhs_sA = yinTA[:, rt * 128:(rt + 1) * 128]
                lhs_tn = yitTn[:, rt * 128:(rt + 1) * 128]
                ps_s = ps_spool.tile([128, CH], f32)
                ps_c = ps_cpool.tile([128, CH], f32)
                # chunk 0 carries the self-block: split the matmuls at the
                # diagonal 128-column range and knock it with an extra
                # accumulated -K*eye matmul (lhsT=I => out += rhs); splits
                # never cross a 512 boundary (PSUM banks)
                if cc == 0:
                    splits = []
                    for h in range(2):
                        h0 = h * 512
                        d0 = rt * 128
                        if h0 <= d0 < h0 + 512:
                            splits += [(h0, d0 - h0, None),
                                       (d0, 128, 'knock'),
                                       (d0 + 128, h0 + 512 - d0 - 128, None)]
                        else:
                            splits.append((h0, 512, None))
                else:
                    splits = [(0, 512, None), (512, 512, None)]
                # all ps_s matmuls first so the s-scan starts early
                for (o, w, kn) in splits:
                    if w <= 0:
                        continue
                    hs = slice(o, o + w)
                    rhs = yinT[:, cc * CH + o: cc * CH + o + w]
                    nc.tensor.matmul(ps_s[:, hs], lhs_s, rhs,
                                     start=True, stop=kn is None)
                    if kn:
                        nc.tensor.matmul(ps_s[:, hs], eyeb[:], knkb_s[:],
                                         start=False, stop=True)
                for (o, w, kn) in splits:
                    if w <= 0:
                        continue
                    hs = slice(o, o + w)
                    rhs = yinT[:, cc * CH + o: cc * CH + o + w]
                    nc.tensor.matmul(ps_c[:, hs], lhs_sA, rhs,
                                     start=True, stop=False)
                    nc.tensor.matmul(ps_c[:, hs], lhs_tn, rhs,
                                     start=False, stop=kn is None)
                    if kn:
                        nc.tensor.matmul(ps_c[:, hs], eyeb[:], knkb_c[:],
                                         start=False, stop=True)
                # per-chunk top-8 candidates
                nc.vector.max(cand_s[:, cc * 8:(cc + 1) * 8], ps_s[:])
                nc.vector.max(cand_c[:, cc * 8:(cc + 1) * 8], ps_c[:])

            def emit_chains(rt, cand_s, cand_c):
                s_k = smpool.tile([128, 16], f32, tag="sk")
                c_k = smpool.tile([128, 16], f32, tag="ck")
                nc.vector.max(s_k[:, 0:8], cand_s[:])
                nc.vector.match_replace(cand_s[:], s_k[:, 0:8], cand_s[:],
                                        NEG)
                nc.vector.max(s_k[:, 8:16], cand_s[:])
                nc.vector.max(c_k[:, 0:8], cand_c[:])
                nc.vector.match_replace(cand_c[:], c_k[:, 0:8], cand_c[:],
                                        NEG)
                nc.vector.max(c_k[:, 8:16], cand_c[:])
                if DBG:
                    nc.sync.dma_start(dbg_sk[:, rt * 16:(rt + 1) * 16],
                                      s_k[:])
                    nc.sync.dma_start(dbg_ck[:, rt * 16:(rt + 1) * 16],
                                      c_k[:])
                return s_k, c_k

            def emit_dis_td():
                # dis(yin_i, yit_i) per local row, from normalized rows:
                # u = yin - yit; dis_td = sqrt(0.25*|u|^2 + eps/4)
                sqtd = smpool.tile([128, NRT], f32, tag="sqtd")
                for jj in range(NRT):
                    u_td = hbig.tile([128, 128], bf16, tag="u_td")
                    nc.gpsimd.tensor_add(u_td[:], rows_yi0[:, jj, :],
                                         rows_ytn[:, jj, :])
                    uscr = hbig.tile([128, 128], f32, tag="uscr")
                    nc.scalar.activation(uscr[:], u_td[:], AF.Square,
                                         accum_out=sqtd[:, jj:jj + 1])
                nc.scalar.activation(dis_td[:], sqtd[:], AF.Sqrt,
                                     scale=0.25, bias=epsqb[:])

            pending = None
            if n_rt >= 2:
                # tiles 0 and 1 run their chunk loops interleaved while the
                # head groups stream in: tile 1 scans earlier chunks while
                # tile 0 waits for later head groups, keeping the DVE busy
                cands01 = []
                for rt in (0, 1):
                    cand_s = candp.tile([128, NCH * 8], f32, tag="cs")
                    cand_c = candp.tile([128, NCH * 8], f32, tag="cc")
                    cands01.append((cand_s, cand_c))
                for cc in range(NCH):
                    if cc >= 1:
                        emit_head_group(pre[cc + 1], cc * 8,
                                        [(yinT, 'pos')], split=2)
                    emit_chunk(0, cc, *cands01[0])
                    emit_chunk(1, cc, *cands01[1])
                emit_dis_td()
                for rt in (0, 1):
                    s_k, c_k = emit_chains(rt, *cands01[rt])
                    if pending is not None:
                        emit_tail(*pending)
                    pending = (rt, s_k, c_k)
                first_rest = 2
            else:
                for cc in range(NCH):
                    if cc >= 1:
                        emit_head_group(pre[cc + 1], cc * 8,
                                        [(yinT, 'pos')], split=2)
                cand_s = candp.tile([128, NCH * 8], f32, tag="cs")
                cand_c = candp.tile([128, NCH * 8], f32, tag="cc")
                for cc in range(NCH):
                    emit_chunk(0, cc, cand_s, cand_c)
                emit_dis_td()
                s_k, c_k = emit_chains(0, cand_s, cand_c)
                pending = (0, s_k, c_k)
                first_rest = 1

            for rt in range(first_rest, n_rt):
                cand_s = candp.tile([128, NCH * 8], f32, tag="cs")
                cand_c = candp.tile([128, NCH * 8], f32, tag="cc")
                for cc in range(NCH):
                    emit_chunk(rt, cc, cand_s, cand_c)
                s_k, c_k = emit_chains(rt, cand_s, cand_c)
                if pending is not None:
                    emit_tail(*pending)
                pending = (rt, s_k, c_k)
            if pending is not None:
                emit_tail(*pending, dve_only=True)

        # ---------------- tail: one DMA of all partials (host sums) ------
        nc.sync.dma_start(out_d[:], eacc[:])

    nc.compile()
    return nc


def kernel(yi: np.ndarray, yi_t: np.ndarray):
    from concourse.bass_utils import run_bass_kernel_spmd

    if "nc" not in _CACHE:
        _CACHE["nc"] = _build_module()
    nc = _CACHE["nc"]

    yi = np.ascontiguousarray(np.asarray(yi, dtype=np.float32))
    yi_t = np.ascontiguousarray(np.asarray(yi_t, dtype=np.float32))
    eye1 = np.eye(128, dtype=np.float32)
    eyek = (KNOCK_S * np.eye(128)).astype(np.float32)
    eyekc = (KNOCK_C * np.eye(128)).astype(np.float32)

    in_maps = []
    for c in range(NCORES):
        lo = c * ROWS
        yi_rot = np.concatenate([yi[lo:], yi[:lo]], axis=0)
        in_maps.append({
            "yi_rot": np.ascontiguousarray(yi_rot),
            "yit_loc": np.ascontiguousarray(yi_t[lo:lo + ROWS]),
            "eye1": eye1,
            "eyek": eyek,
            "eyekc": eyekc,
        })

    res = run_bass_kernel_spmd(nc, in_maps, list(range(NCORES))).results

    e1 = np.float64(0.0)
    e2 = np.float64(0.0)
    for c in range(NCORES):
        out = res[c]["out"]
        e1 += out[:, 0:NRT].astype(np.float64).sum()
        e2 += out[:, NRT:2 * NRT].astype(np.float64).sum()
    e1 = np.float32(e1)
    e2 = np.float32(e2)
    return (np.float32(e1 + e2), e1, e2)
